# revision 76
# baseline (speedup 1.0000x reference)
"""Trainium2 Bass kernel for nn_ConditionalSplineFlow (8-core data parallel).

Layout strategy:
  - MLP runs in "transposed world": activations [feature, rows] so weight
    matrices act as lhsT directly; final GEMM flips orientation using h2^T as
    lhsT, giving params [rows(part), 1472(free)] with W3 columns reordered to
    [uw(k-major,d-inner) | uh | ud(j-major)].
  - Bin space is batched per sub-chunk of RT row-tiles with uw/uh columns
    interleaved (k, d, q) so the (cumw, cumh) edge pair for one dim sits in
    adjacent f16 elements: the bin-search walk moves both via one f32-bitcast
    predicated copy (half the element count). Cumsum chain is an unrolled f16
    DVE tensor-add chain (2x perf mode), 1/S comes from ACT ln/exp
    (exp(-lnS + ln(SCL)) = SCL/S), edges are two batched f16 DVE tensor ops.
  - Spline formula is elementwise in (row, dim), chunked per sub-chunk.
  - LU layer folded to  x' = y @ (L@U)^T + b  via PE transpose + matmul.
  - Per-layer logdet of LU and the gaussian constant are folded on host.
"""
import os
import numpy as np
from contextlib import ExitStack

import concourse.bass as bass
import concourse.bacc as bacc
import concourse.tile as tile
import concourse.mybir as mybir
from concourse import bass_utils
from concourse.masks import make_identity

# Pin all activations to the one table set that covers Exp/Ln/Relu/Copy/Abs —
# the default per-function chooser ping-pongs between sets (~2.6us per swap,
# once per row-tile). Masking the other sets (order preserved, so positional
# set ids stay valid) forces a single resident table.
_PINNED_ACT_SET = "natural_log_exp_and_others"
_orig_gat = bacc.get_activation_tables


def _gat_pinned(arch):
    tabs = _orig_gat(arch)
    return {name: (fns if name == _PINNED_ACT_SET else set())
            for name, fns in tabs.items()}


bacc.get_activation_tables = _gat_pinned

F32 = mybir.dt.float32
F16 = mybir.dt.float16
U8 = mybir.dt.uint8
AF = mybir.ActivationFunctionType
OP = mybir.AluOpType

D = 64
NB = 8
L = 5
HID = 256
ODIM = 1472
BND = 5.0
MIN_W = 1e-3
MIN_D = 1e-3
SCL = 2.0 * BND * (1.0 - MIN_W * NB)      # 9.92
SPBOUND = 1.0 - MIN_D                      # softplus value at padded boundary
N_CORES = 8
BATCH = 32768

MM_DT = F16   # matmul dtype for MLP
BT = F16      # bin-space dtype (V / Gc / chain / edges)


def build_program(rpc, has_b3=False, has_lub=True, ablate=()):
    """Build the single-core program for `rpc` rows. Returns nc.

    `ablate` (dev-only; never set by kernel()): drop pipeline sections to
    isolate their HW cost. Subsets of {"exps", "binning", "formula", "lu"}.
    Later sections imply earlier ones stay: ablating "exps" also ablates
    "binning"/"formula"/"lu" (they consume its outputs)."""
    A = set(ablate)
    if "exps" in A:
        A |= {"binning"}
    if "binning" in A:
        A |= {"formula"}
    if "formula" in A:
        A |= {"lu"}
    nc = bacc.Bacc(
        "TRN2", target_bir_lowering=False, debug=False,
        enable_asserts=False, num_devices=N_CORES,
    )
    NT = rpc // 128               # row tiles
    CHW = min(rpc, int(os.environ.get("SPLINE_CHW", "1024")))  # mlp chunk rows
    NCH = rpc // CHW              # chunks
    RT_PER_CH = CHW // 128
    RT = int(os.environ.get("SPLINE_RT", "4"))   # bin sub-chunk (row tiles)
    assert RT_PER_CH % RT == 0
    NLNSCL = float(np.log(SCL))

    # ---------------- DRAM I/O ----------------
    xin_d = nc.dram_tensor("xin", [rpc, D], F32, kind="ExternalInput").ap()
    ctxT_d = nc.dram_tensor("ctxT", [128, rpc], MM_DT, kind="ExternalInput").ap()
    w1_d = nc.dram_tensor("w1", [L, 128, HID], MM_DT, kind="ExternalInput").ap()
    w2_d = nc.dram_tensor("w2", [L, 2, 128, HID], MM_DT, kind="ExternalInput").ap()
    w3_d = nc.dram_tensor("w3", [L, 2, 128, ODIM], MM_DT, kind="ExternalInput").ap()
    b1_d = nc.dram_tensor("bias1", [L, 2, 128], F32, kind="ExternalInput").ap()
    b2_d = nc.dram_tensor("bias2", [L, 2, 128], F32, kind="ExternalInput").ap()
    at_d = nc.dram_tensor("at", [L, D, D], MM_DT, kind="ExternalInput").ap()
    if has_lub:
        lub_d = nc.dram_tensor("lub", [L, D], F32, kind="ExternalInput").ap()
    ce_d = nc.dram_tensor("cedge", [7, 2 * D], BT, kind="ExternalInput").ap()
    if has_b3:
        b3_d = nc.dram_tensor("bias3", [1, ODIM], F32, kind="ExternalInput").ap()
    out_d = nc.dram_tensor("out", [rpc], F32, kind="ExternalOutput").ap()

    with tile.TileContext(nc) as tc, ExitStack() as ctx:
        # ---------------- pools ----------------
        singles = ctx.enter_context(tc.tile_pool(name="singles", bufs=1))
        wpool = ctx.enter_context(tc.tile_pool(name="wpool", bufs=2))
        hpool = ctx.enter_context(tc.tile_pool(name="hpool", bufs=2))
        binp = ctx.enter_context(tc.tile_pool(name="binp", bufs=2))
        frm = ctx.enter_context(tc.tile_pool(
            name="frm", bufs=int(os.environ.get("SPLINE_FRM_BUFS", "2"))))
        ps_par = ctx.enter_context(tc.tile_pool(
            name="ps_par", bufs=int(os.environ.get("SPLINE_PSPAR_BUFS", "1")),
            space="PSUM"))
        ps_h = ctx.enter_context(tc.tile_pool(
            name="ps_h", bufs=int(os.environ.get("SPLINE_PSH_BUFS", "1")),
            space="PSUM"))
        ps_lu = ctx.enter_context(tc.tile_pool(
            name="ps_lu", bufs=int(os.environ.get("SPLINE_PSLU_BUFS", "1")),
            space="PSUM"))

        # ---------------- resident tiles ----------------
        ident = singles.tile([128, 128], F32)
        make_identity(nc, ident)
        ctxT = singles.tile([128, rpc], MM_DT)
        nc.sync.dma_start(out=ctxT, in_=ctxT_d)
        # x ping-pong, resident across a layer
        xa = singles.tile([128, NT, D], F32, tag="xa")
        xb = singles.tile([128, NT, D], F32, tag="xb")
        nc.sync.dma_start(out=xa, in_=xin_d.rearrange("(t p) d -> p t d", p=128))
        # per-dim logdet accumulator [128, NT, D]
        ldacc = singles.tile([128, NT, D], F32, tag="ldacc")
        if A:
            nc.vector.memset(ldacc, 0.0)
        # bias const for invs = exp(-lnS + ln(SCL))
        lnscl_t = singles.tile([128, 1], F32)
        nc.vector.memset(lnscl_t, NLNSCL)
        # edge constants c_j, duplicated per (d, q) pair: [128, 7, 128]
        cedge = singles.tile([128, 7, 2 * D], BT)
        nc.sync.dma_start(
            out=cedge,
            in_=bass.AP(tensor=ce_d.tensor, offset=0,
                        ap=[[0, 128], [2 * D, 7], [1, 2 * D]]),
        )
        if has_b3:
            ones1 = singles.tile([1, 128], F32)
            nc.vector.memset(ones1, 1.0)
            b3sb = singles.tile([1, ODIM], F32)
            nc.sync.dma_start(out=b3sb, in_=b3_d)
        # persistent V tiles: 6 packed f16 lanes per (slot j, dim d):
        # (e_j, ch_j, e_{j+1}, ch_{j+1}, sp_j, sp_{j+1}) = 3 f32 pairs, so the
        # walk moves left+right values of all planes in ONE f32-bitcast
        # predicated copy. Slots 0..7 (bin index k). Boundary constants
        # written once.
        NVB = int(os.environ.get("SPLINE_NVB", "2"))
        V6bufs = [singles.tile([128, RT, 8, 3, 2 * D], BT, name=f"V6b{i}")
                  for i in range(NVB)]
        for i in range(NVB):
            v = V6bufs[i].rearrange("p r s t (d q) -> p r s t d q", q=2)
            nc.gpsimd.memset(v[:, :, 0, 0, :, :], -BND)
            nc.gpsimd.memset(v[:, :, 7, 1, :, :], BND)
            nc.gpsimd.memset(v[:, :, 0, 2, :, 0], SPBOUND)
            nc.gpsimd.memset(v[:, :, 7, 2, :, 1], SPBOUND)

        x_cur, x_nxt = xa, xb

        for l in range(L):
            # ---------------- layer weights ----------------
            w1t = wpool.tile([128, HID], MM_DT, tag="w1")
            w2t = wpool.tile([128, 2, HID], MM_DT, tag="w2")
            w3t = wpool.tile([128, 2, ODIM], MM_DT, tag="w3")
            b1t = wpool.tile([128, 2], F32, tag="b1")
            b2t = wpool.tile([128, 2], F32, tag="b2")
            att = wpool.tile([64, D], MM_DT, tag="at")
            if has_lub:
                lubt = wpool.tile([128, D], F32, tag="lub")
            nc.sync.dma_start(out=w1t, in_=w1_d[l])
            nc.sync.dma_start(out=w2t, in_=w2_d[l].rearrange("k p h -> p k h"))
            nc.sync.dma_start(out=w3t, in_=w3_d[l].rearrange("k p h -> p k h"))
            nc.sync.dma_start(out=b1t, in_=b1_d[l].rearrange("t p -> p t"))
            nc.sync.dma_start(out=b2t, in_=b2_d[l].rearrange("t p -> p t"))
            nc.sync.dma_start(out=att, in_=at_d[l])
            if has_lub:
                nc.sync.dma_start(
                    out=lubt,
                    in_=bass.AP(tensor=lub_d.tensor, offset=l * D,
                                ap=[[0, 128], [1, D]]),
                )

            for chi in range(NCH):
                # ---------------- MLP chunk (transposed world) ----------------
                c0 = chi * CHW
                NHALF = max(1, CHW // 512)
                h1t = hpool.tile([128, 2, CHW], MM_DT, tag="h1")
                for m in range(2):
                    ps1 = ps_h.tile([128, CHW], F32, tag="psh")
                    for hf in range(NHALF):
                        h0 = hf * 512
                        hw_ = min(512, CHW - h0)
                        nc.tensor.matmul(
                            ps1[:, h0:h0 + hw_],
                            lhsT=w1t[:, m * 128:(m + 1) * 128],
                            rhs=ctxT[:, c0 + h0:c0 + h0 + hw_])
                    nc.scalar.activation(h1t[:, m, :], ps1, AF.Relu,
                                         bias=b1t[:, m:m + 1])
                h2t = hpool.tile([128, 2, CHW], MM_DT, tag="h2")
                for m in range(2):
                    ps2 = ps_h.tile([128, CHW], F32, tag="psh")
                    for hf in range(NHALF):
                        h0 = hf * 512
                        hw_ = min(512, CHW - h0)
                        for kk in range(2):
                            nc.tensor.matmul(
                                ps2[:, h0:h0 + hw_],
                                lhsT=w2t[:, kk, m * 128:(m + 1) * 128],
                                rhs=h1t[:, kk, h0:h0 + hw_],
                                start=(kk == 0), stop=(kk == 1))
                    nc.scalar.activation(h2t[:, m, :], ps2, AF.Relu,
                                         bias=b2t[:, m:m + 1])

                if "binning" not in A:
                    xcc_nxt = frm.tile([128, RT, D], BT, tag="xcc", name="xcc")
                    nc.gpsimd.tensor_scalar(
                        xcc_nxt,
                        x_cur[:, chi * RT_PER_CH:chi * RT_PER_CH + RT, :],
                        -BND, BND, OP.max, OP.min)
                for sub in range(RT_PER_CH // RT):
                    rt0 = chi * RT_PER_CH + sub * RT
                    V6 = V6bufs[(rt0 // RT) % NVB]
                    if "exps" not in A:
                        # exp(params) tile, one ACT write per row-tile:
                        # cols 0:1024 = uw/uh [8(k), 64(d), 2(q)] (cumsummed in
                        # place below), cols 1024:1472 = ud [7, 64].
                        EP = binp.tile([128, RT, ODIM], BT, tag="EP", name="EP")
                        E = EP[:, :, 0:1024].rearrange(
                            "p r (k d q) -> p r k d q", k=NB, d=D, q=2)
                        EU = EP[:, :, 1024:1472].rearrange(
                            "p r (j d) -> p r j d", j=7, d=D)
                    if "binning" not in A:
                        Gc6 = binp.tile([128, RT, 3, 2 * D], BT, tag="Gc6",
                                        name="Gc6")
                        xcc = xcc_nxt
                        if sub + 1 < RT_PER_CH // RT:
                            # prefetch next sub-chunk's clip ahead of this
                            # sub-chunk's formula ops in Pool's in-order queue
                            xcc_nxt = frm.tile([128, RT, D], BT, tag="xcc",
                                               name="xcc")
                            nc.gpsimd.tensor_scalar(
                                xcc_nxt, x_cur[:, rt0 + RT:rt0 + 2 * RT, :],
                                -BND, BND, OP.max, OP.min)
                    if A:
                        rq = frm.tile([128, RT], F32, tag="rq", name="rq")

                    for rti in range(RT):
                        rt = rt0 + rti
                        r0 = (sub * RT + rti) * 128
                        # -------- GEMM3: params [128 rows, 1472] --------
                        psp = ps_par.tile([128, ODIM], F32, tag="pspar")
                        nslices = [(0, 512), (512, 512), (1024, 448)]
                        for (ns, nw) in nslices:
                            for kk in range(2):
                                nc.tensor.matmul(
                                    psp[:, ns:ns + nw],
                                    lhsT=h2t[:, kk, r0:r0 + 128],
                                    rhs=w3t[:, kk, ns:ns + nw],
                                    start=(kk == 0),
                                    stop=(kk == 1) if not has_b3 else False,
                                )
                            if has_b3:
                                nc.tensor.matmul(
                                    psp[:, ns:ns + nw], lhsT=ones1,
                                    rhs=b3sb[:, ns:ns + nw],
                                    start=False, stop=True)

                        if "exps" in A:
                            nc.vector.tensor_reduce(
                                rq[:, rti:rti + 1],
                                psp[:, 0:64].unsqueeze(1),
                                mybir.AxisListType.X, OP.add)
                            continue
                        # -------- exp / softplus (ACT) --------
                        nc.scalar.activation(EP[:, rti], psp, AF.Exp)
                        # softplus(ud) = Ln(exp(ud) + 1) into both sp lanes:
                        # lane 0 at slots 1..7 (sp_j), lane 1 at slots 0..6
                        # (sp_{j+1})
                        vsp = V6[:, rti].rearrange(
                            "p s t (d q) -> p s t d q", q=2)
                        nc.scalar.activation(vsp[:, 1:8, 2, :, 0], EU[:, rti],
                                             AF.Ln, bias=1.0)
                        nc.scalar.activation(vsp[:, 0:7, 2, :, 1], EU[:, rti],
                                             AF.Ln, bias=1.0)

                    if "binning" in A:
                        nc.vector.tensor_reduce(
                            rq, E[:, :, 0, :, 0], mybir.AxisListType.X, OP.add)
                        if l == 0:
                            nc.vector.tensor_copy(ldacc[:, rt0:rt0 + RT, 0:1],
                                                  rq.unsqueeze(2))
                        else:
                            nc.vector.tensor_add(
                                ldacc[:, rt0:rt0 + RT, 0:1], rq.unsqueeze(2),
                                ldacc[:, rt0:rt0 + RT, 0:1])
                        continue

                    # -------- cumsum chain, in place on E (f16, DVE) --------
                    Em = E.rearrange("p r k d q -> p r k (d q)")
                    for j in range(1, NB):
                        nc.vector.tensor_add(Em[:, :, j, :], Em[:, :, j - 1, :],
                                             Em[:, :, j, :])
                    # -------- invs = SCL / S via ACT ln/exp --------
                    lnS = frm.tile([128, RT, 2 * D], F32, tag="lnS", name="lnS")
                    nc.scalar.activation(lnS, Em[:, :, NB - 1, :], AF.Ln)
                    invs = frm.tile([128, RT, 2 * D], BT, tag="invs",
                                    name="invs")
                    nc.scalar.activation(invs, lnS, AF.Exp, bias=lnscl_t,
                                         scale=-1.0)
                    # -------- edges into packed V pair 0 (f16, DVE) --------
                    V6p0 = V6[:, :, 1:8, 0, :]
                    nc.vector.tensor_mul(
                        V6p0, Em[:, :, 0:7, :],
                        invs.unsqueeze(2).to_broadcast([128, RT, 7, 2 * D]))
                    nc.vector.tensor_add(
                        V6p0, V6p0,
                        cedge.unsqueeze(1).to_broadcast([128, RT, 7, 2 * D]))
                    # right-neighbor (e, ch) duplicated into pair 1 (TC 4x)
                    nc.vector.tensor_copy(V6[:, :, 0:7, 1, :], V6p0)

                    # -------- masks (f16, 2x) + single packed walk (DVE) -----
                    su = binp.tile([128, RT, 7, D], BT, tag="su", name="su")
                    nc.vector.tensor_tensor(
                        su,
                        xcc.unsqueeze(2).to_broadcast([128, RT, 7, D]),
                        V6[:, :, 1:8, 0, :].rearrange(
                            "p r s (d q) -> p r s d q", q=2)[:, :, :, :, 0],
                        OP.is_ge)
                    V632 = V6.bitcast(F32)      # [128, RT, 8, 3, 64]
                    Gc632 = Gc6.bitcast(F32)    # [128, RT, 3, 64]
                    nc.vector.tensor_copy(
                        Gc6.rearrange("p r t m -> p r (t m)"),
                        V6[:, :, 0, :, :].rearrange("p r t m -> p r (t m)"))
                    su16 = su.bitcast(mybir.dt.uint16)
                    for j in range(1, 8):
                        nc.vector.copy_predicated(
                            Gc632,
                            su16[:, :, j - 1:j, :].to_broadcast(
                                [128, RT, 3, D]),
                            V632[:, :, j, :, :])

                    if "formula" in A:
                        nc.vector.tensor_reduce(
                            rq,
                            Gc6[:, :, 0, :].rearrange(
                                "p r (d q) -> p r d q", q=2)[:, :, :, 0],
                            mybir.AxisListType.X, OP.add)
                        if l == 0:
                            nc.vector.tensor_copy(ldacc[:, rt0:rt0 + RT, 0:1],
                                                  rq.unsqueeze(2))
                        else:
                            nc.vector.tensor_add(
                                ldacc[:, rt0:rt0 + RT, 0:1], rq.unsqueeze(2),
                                ldacc[:, rt0:rt0 + RT, 0:1])
                        continue
                    # ------------- formula (chunked, mostly f16) -------------
                    FSH = [128, RT, D]
                    G0 = Gc6[:, :, 0, :].rearrange("p r (d q) -> p r d q", q=2)
                    G1 = Gc6[:, :, 1, :].rearrange("p r (d q) -> p r d q", q=2)
                    le, lch = G0[:, :, :, 0], G0[:, :, :, 1]
                    re_, rch = G1[:, :, :, 0], G1[:, :, :, 1]
                    Gdp = frm.tile([128, RT, 2 * D], BT, tag="Gdp", name="Gdp")
                    nc.vector.tensor_scalar(Gdp, Gc6[:, :, 2, :],
                                            MIN_D, None, OP.add)
                    Gdpq = Gdp.rearrange("p r (d q) -> p r d q", q=2)
                    ind, ind1 = Gdpq[:, :, :, 0], Gdpq[:, :, :, 1]

                    def ft(tag, dt_=BT):
                        return frm.tile(FSH, dt_, tag=tag, name=tag)

                    xt = x_cur[:, rt0:rt0 + RT, :]
                    # 1/in_w and 1/denom via ACT ln/exp keeps the mults f16-2x
                    in_w = ft("in_w"); nc.gpsimd.tensor_sub(in_w, re_, le)
                    lnw = ft("lnw")
                    nc.scalar.activation(lnw, in_w, AF.Ln)
                    rw = ft("rw")
                    nc.scalar.activation(rw, lnw, AF.Exp, scale=-1.0)
                    tnum = ft("tnum"); nc.gpsimd.tensor_sub(tnum, xcc, le)
                    th = ft("th"); nc.vector.tensor_mul(th, tnum, rw)
                    in_h = ft("in_h"); nc.gpsimd.tensor_sub(in_h, rch, lch)
                    idel = ft("idel"); nc.vector.tensor_mul(idel, in_h, rw)
                    # squares on ACT; (1-th)^2 fused via scale/bias
                    th2 = ft("th2"); nc.scalar.activation(th2, th, AF.Square)
                    omt2 = ft("omt2")
                    nc.scalar.activation(omt2, th, AF.Square, bias=1.0, scale=-1.0)
                    idel2 = ft("idel2")
                    nc.scalar.activation(idel2, idel, AF.Square)
                    tomt = ft("tomt"); nc.vector.tensor_sub(tomt, th, th2)
                    t1 = ft("t1"); nc.vector.tensor_mul(t1, idel, th2)
                    t2 = ft("t2"); nc.vector.tensor_mul(t2, ind, tomt)
                    nsum = t1; nc.vector.tensor_add(nsum, t1, t2)
                    numer = in_h; nc.vector.tensor_mul(numer, in_h, nsum)
                    dd = ft("dd"); nc.gpsimd.tensor_add(dd, ind, ind1)
                    dd2 = ft("dd2")
                    nc.vector.scalar_tensor_tensor(dd2, idel, -2.0, dd,
                                                   OP.mult, OP.add)
                    dt = dd2; nc.vector.tensor_mul(dt, dd2, tomt)
                    denom = ft("denom")
                    nc.gpsimd.tensor_add(denom, idel, dt)
                    lnden = ft("lnden")
                    nc.scalar.activation(lnden, denom, AF.Ln)
                    rden = ft("rden")
                    nc.scalar.activation(rden, lnden, AF.Exp, scale=-1.0)
                    yq = rden; nc.vector.tensor_mul(yq, numer, rden)
                    y = ft("y", F32); nc.gpsimd.tensor_add(y, lch, yq)
                    u1 = ft("u1"); nc.vector.tensor_mul(u1, th2, ind1)
                    idt = ft("idt"); nc.vector.tensor_mul(idt, idel, tomt)
                    u2 = u1
                    nc.vector.scalar_tensor_tensor(u2, idt, 2.0, u1,
                                                   OP.mult, OP.add)
                    u3 = ft("u3"); nc.gpsimd.tensor_mul(u3, ind, omt2)
                    uu = u2; nc.vector.tensor_add(uu, u2, u3)
                    dnum = idel2; nc.vector.tensor_mul(dnum, uu, idel2)
                    lnd = ft("lnd"); nc.scalar.activation(lnd, dnum, AF.Ln)
                    ldt = ft("ldt")
                    nc.vector.scalar_tensor_tensor(ldt, lnden, -2.0, lnd,
                                                   OP.mult, OP.add)
                    # inside mask + select (f16 mask, bitcast for predication)
                    absx = ft("absx")
                    nc.scalar.activation(absx, xt, AF.Abs)
                    insu = frm.tile([128, RT, D], BT, tag="insu", name="insu")
                    nc.vector.tensor_scalar(insu, absx, BND, None, OP.is_le)
                    yfin = ft("yfin", F32)
                    nc.gpsimd.tensor_copy(yfin, xt)
                    nc.vector.copy_predicated(yfin, insu.bitcast(mybir.dt.uint16),
                                              y)
                    # masked per-dim logdet accumulate, chained across layers
                    ldm = ft("ldm")
                    nc.vector.tensor_mul(ldm, ldt, insu)
                    if l == 0:
                        nc.gpsimd.tensor_copy(ldacc[:, rt0:rt0 + RT, :], ldm)
                    else:
                        nc.gpsimd.tensor_add(ldacc[:, rt0:rt0 + RT, :], ldm,
                                             ldacc[:, rt0:rt0 + RT, :])

                    # -------- LU per row-tile: x' = y @ A^T + b --------
                    if "lu" in A:
                        continue
                    for rti in range(RT):
                        rt = rt0 + rti
                        pst = ps_lu.tile([64, 128], F32, tag="pst")
                        nc.tensor.transpose(pst, yfin[:, rti, :], ident)
                        yT = frm.tile([64, 128], MM_DT, tag="yT", name="yT")
                        nc.scalar.copy(yT, pst)
                        psx = ps_lu.tile([128, D], F32, tag="psx")
                        nc.tensor.matmul(psx, lhsT=yT, rhs=att)
                        if has_lub:
                            nc.vector.tensor_add(x_nxt[:, rt, :], psx, lubt)
                        else:
                            nc.scalar.copy(x_nxt[:, rt, :], psx)


            if "lu" not in A:
                x_cur, x_nxt = x_nxt, x_cur

        # ---------------- final: out = -0.5*sum(x^2) + sum_d ld + const -----
        xsq = singles.tile([128, NT, D], F32)
        nc.vector.tensor_mul(xsq, x_cur, x_cur)
        ov2 = singles.tile([128, NT, D], F32)
        nc.vector.scalar_tensor_tensor(ov2, xsq, -0.5, ldacc, OP.mult, OP.add)
        ov = singles.tile([128, NT], F32)
        nc.vector.tensor_reduce(ov, ov2, mybir.AxisListType.X, OP.add)
        # const added on host (exact); DMA out
        nc.sync.dma_start(out=out_d.rearrange("(t p) -> p t", p=128), in_=ov)

    nc.compile()
    return nc


# ------------------------- host side -------------------------

def _host_prep(inputs):
    x = np.ascontiguousarray(inputs["inputs"].astype(np.float32))
    ctx = inputs["context"].astype(np.float32)
    W1 = inputs["W1"].astype(np.float32)
    W2 = inputs["W2"].astype(np.float32)
    W3 = inputs["W3"].astype(np.float32)
    b1 = inputs["b1"].astype(np.float32)
    b2 = inputs["b2"].astype(np.float32)
    b3 = inputs["b3"].astype(np.float32)

    mmnp = np.float16 if MM_DT == F16 else np.float32

    cols = np.arange(D * 23).reshape(D, 23)
    # uw/uh interleaved (k, d, q): the (cumw, cumh) pair for one dim lands in
    # adjacent f16 elements (walked as one f32); ud stays (j, d).
    wh = np.stack([cols[:, 0:8], cols[:, 8:16]], axis=-1)  # [d, k, 2]
    perm = np.concatenate([
        wh.transpose(1, 0, 2).reshape(-1),  # (k, d, q)
        cols[:, 16:23].T.reshape(-1),
    ])
    W3r = W3[:, :, perm]
    b3r = b3[:, perm]
    has_b3 = bool(np.any(b3r != 0.0))

    ctxT = np.ascontiguousarray(ctx.T.astype(mmnp))                 # [128, B]
    w1 = np.ascontiguousarray(W1.astype(mmnp))                      # [L,128,256]
    w2 = np.ascontiguousarray(
        W2.reshape(L, 2, 128, HID).astype(mmnp))                    # [L,2,128,256]
    w3 = np.ascontiguousarray(
        W3r.reshape(L, 2, 128, ODIM).astype(mmnp))                  # [L,2,128,1472]
    bias1 = np.ascontiguousarray(b1.reshape(L, 2, 128))
    bias2 = np.ascontiguousarray(b2.reshape(L, 2, 128))

    li = np.tril_indices(D, -1)
    ui = np.triu_indices(D, 1)
    at = np.zeros((L, D, D), np.float32)
    lld = 0.0
    for l in range(L):
        Lm = np.eye(D, dtype=np.float64)
        Lm[li] = inputs["lower_entries"][l].astype(np.float64)
        diag = np.log1p(np.exp(inputs["upper_diag"][l].astype(np.float64))) + 1e-3
        Um = np.zeros((D, D), np.float64)
        Um[ui] = inputs["upper_entries"][l].astype(np.float64)
        Um += np.diag(diag)
        at[l] = (Lm @ Um).T.astype(np.float32)  # cast to mm dtype below
        lld += float(np.sum(np.log(diag)))
    lub = np.ascontiguousarray(inputs["lu_bias"].astype(np.float32))

    cj = ((2.0 * BND * MIN_W) * np.arange(1, 8).astype(np.float32) - BND)
    btnp = np.float16 if BT == F16 else np.float32
    cedge = np.ascontiguousarray(
        np.broadcast_to(cj[:, None], (7, 2 * D)).astype(btnp))

    const_total = -0.5 * D * float(np.log(2.0 * np.pi)) + lld

    has_lub = bool(np.any(lub != 0.0))
    shared = dict(w1=w1, w2=w2, w3=w3, bias1=bias1, bias2=bias2,
                  at=at.astype(mmnp), cedge=cedge)
    if has_lub:
        shared["lub"] = lub
    if has_b3:
        shared["bias3"] = np.ascontiguousarray(b3r.reshape(1, ODIM))
    return x, ctxT, shared, has_b3, has_lub, const_total


_CACHE = {}


def kernel(**inputs):
    rpc = inputs["inputs"].shape[0] // N_CORES
    x, ctxT, shared, has_b3, has_lub, const_total = _host_prep(inputs)

    key = (rpc, has_b3, has_lub)
    if key not in _CACHE:
        _CACHE[key] = build_program(rpc, has_b3=has_b3, has_lub=has_lub)
    nc = _CACHE[key]

    in_maps = []
    for c in range(N_CORES):
        m = dict(shared)
        m["xin"] = np.ascontiguousarray(x[c * rpc:(c + 1) * rpc])
        m["ctxT"] = np.ascontiguousarray(ctxT[:, c * rpc:(c + 1) * rpc])
        in_maps.append(m)

    res = bass_utils.run_bass_kernel_spmd(nc, in_maps, core_ids=list(range(N_CORES)))
    out = np.concatenate([r["out"] for r in res.results])
    return (out + np.float32(const_total)).astype(np.float32)


# revision 78
# speedup vs baseline: 1.0089x; 1.0089x over previous
"""Trainium2 Bass kernel for nn_ConditionalSplineFlow (8-core data parallel).

Layout strategy:
  - MLP runs in "transposed world": activations [feature, rows] so weight
    matrices act as lhsT directly; final GEMM flips orientation using h2^T as
    lhsT, giving params [rows(part), 1472(free)] with W3 columns reordered to
    [uw(k-major,d-inner) | uh | ud(j-major)].
  - Bin space is batched per sub-chunk of RT row-tiles with uw/uh columns
    interleaved (k, d, q) so the (cumw, cumh) edge pair for one dim sits in
    adjacent f16 elements: the bin-search walk moves both via one f32-bitcast
    predicated copy (half the element count). Cumsum chain is an unrolled f16
    DVE tensor-add chain (2x perf mode), 1/S comes from ACT ln/exp
    (exp(-lnS + ln(SCL)) = SCL/S), edges are two batched f16 DVE tensor ops.
  - Spline formula is elementwise in (row, dim), chunked per sub-chunk.
  - LU layer folded to  x' = y @ (L@U)^T + b  via PE transpose + matmul.
  - Per-layer logdet of LU and the gaussian constant are folded on host.
"""
import os
import numpy as np
from contextlib import ExitStack

import concourse.bass as bass
import concourse.bacc as bacc
import concourse.tile as tile
import concourse.mybir as mybir
from concourse import bass_utils
from concourse.masks import make_identity

# Pin all activations to the one table set that covers Exp/Ln/Relu/Copy/Abs —
# the default per-function chooser ping-pongs between sets (~2.6us per swap,
# once per row-tile). Masking the other sets (order preserved, so positional
# set ids stay valid) forces a single resident table.
_PINNED_ACT_SET = "natural_log_exp_and_others"
_orig_gat = bacc.get_activation_tables


def _gat_pinned(arch):
    tabs = _orig_gat(arch)
    return {name: (fns if name == _PINNED_ACT_SET else set())
            for name, fns in tabs.items()}


bacc.get_activation_tables = _gat_pinned

F32 = mybir.dt.float32
F16 = mybir.dt.float16
U8 = mybir.dt.uint8
AF = mybir.ActivationFunctionType
OP = mybir.AluOpType

D = 64
NB = 8
L = 5
HID = 256
ODIM = 1472
BND = 5.0
MIN_W = 1e-3
MIN_D = 1e-3
SCL = 2.0 * BND * (1.0 - MIN_W * NB)      # 9.92
SPBOUND = 1.0 - MIN_D                      # softplus value at padded boundary
N_CORES = 8
BATCH = 32768

MM_DT = F16   # matmul dtype for MLP
BT = F16      # bin-space dtype (V / Gc / chain / edges)


def build_program(rpc, has_b3=False, has_lub=True, ablate=()):
    """Build the single-core program for `rpc` rows. Returns nc.

    `ablate` (dev-only; never set by kernel()): drop pipeline sections to
    isolate their HW cost. Subsets of {"exps", "binning", "formula", "lu"}.
    Later sections imply earlier ones stay: ablating "exps" also ablates
    "binning"/"formula"/"lu" (they consume its outputs)."""
    A = set(ablate)
    if "exps" in A:
        A |= {"binning"}
    if "binning" in A:
        A |= {"formula"}
    if "formula" in A:
        A |= {"lu"}
    nc = bacc.Bacc(
        "TRN2", target_bir_lowering=False, debug=False,
        enable_asserts=False, num_devices=N_CORES,
    )
    NT = rpc // 128               # row tiles
    CHW = min(rpc, int(os.environ.get("SPLINE_CHW", "1024")))  # mlp chunk rows
    NCH = rpc // CHW              # chunks
    RT_PER_CH = CHW // 128
    RT = int(os.environ.get("SPLINE_RT", "4"))   # bin sub-chunk (row tiles)
    assert RT_PER_CH % RT == 0
    NLNSCL = float(np.log(SCL))

    # ---------------- DRAM I/O ----------------
    xin_d = nc.dram_tensor("xin", [rpc, D], F32, kind="ExternalInput").ap()
    ctxT_d = nc.dram_tensor("ctxT", [128, rpc], MM_DT, kind="ExternalInput").ap()
    w1_d = nc.dram_tensor("w1", [L, 128, HID], MM_DT, kind="ExternalInput").ap()
    w2_d = nc.dram_tensor("w2", [L, 2, 128, HID], MM_DT, kind="ExternalInput").ap()
    w3_d = nc.dram_tensor("w3", [L, 2, 128, ODIM], MM_DT, kind="ExternalInput").ap()
    b1_d = nc.dram_tensor("bias1", [L, 2, 128], F32, kind="ExternalInput").ap()
    b2_d = nc.dram_tensor("bias2", [L, 2, 128], F32, kind="ExternalInput").ap()
    at_d = nc.dram_tensor("at", [L, D, D], MM_DT, kind="ExternalInput").ap()
    if has_lub:
        lub_d = nc.dram_tensor("lub", [L, D], F32, kind="ExternalInput").ap()
    ce_d = nc.dram_tensor("cedge", [7, 2 * D], BT, kind="ExternalInput").ap()
    if has_b3:
        b3_d = nc.dram_tensor("bias3", [1, ODIM], F32, kind="ExternalInput").ap()
    out_d = nc.dram_tensor("out", [rpc], F32, kind="ExternalOutput").ap()

    with tile.TileContext(nc) as tc, ExitStack() as ctx:
        # ---------------- pools ----------------
        singles = ctx.enter_context(tc.tile_pool(name="singles", bufs=1))
        wpool = ctx.enter_context(tc.tile_pool(name="wpool", bufs=2))
        hpool = ctx.enter_context(tc.tile_pool(name="hpool", bufs=2))
        binp = ctx.enter_context(tc.tile_pool(name="binp", bufs=2))
        frm = ctx.enter_context(tc.tile_pool(
            name="frm", bufs=int(os.environ.get("SPLINE_FRM_BUFS", "2"))))
        ps_par = ctx.enter_context(tc.tile_pool(
            name="ps_par", bufs=int(os.environ.get("SPLINE_PSPAR_BUFS", "1")),
            space="PSUM"))
        ps_h = ctx.enter_context(tc.tile_pool(
            name="ps_h", bufs=int(os.environ.get("SPLINE_PSH_BUFS", "1")),
            space="PSUM"))
        ps_lu = ctx.enter_context(tc.tile_pool(
            name="ps_lu", bufs=int(os.environ.get("SPLINE_PSLU_BUFS", "1")),
            space="PSUM"))

        # ---------------- resident tiles ----------------
        ident = singles.tile([128, 128], F32)
        make_identity(nc, ident)
        ctxT = singles.tile([128, rpc], MM_DT)
        nc.sync.dma_start(out=ctxT, in_=ctxT_d)
        # PE p-state warmup: ~3us of back-to-back dummy matmuls during the
        # initial DMA window so the first real GEMMs run at full clock
        NWARM = int(os.environ.get("SPLINE_WARMUP_MM", "24"))
        if NWARM:
            ps_warm = ps_h.tile([128, CHW], F32, tag="psh")
            for _ in range(NWARM):
                nc.tensor.matmul(ps_warm[:, 0:128], lhsT=ident, rhs=ident)
        # x ping-pong, resident across a layer
        xa = singles.tile([128, NT, D], F32, tag="xa")
        xb = singles.tile([128, NT, D], F32, tag="xb")
        nc.sync.dma_start(out=xa, in_=xin_d.rearrange("(t p) d -> p t d", p=128))
        # per-dim logdet accumulator [128, NT, D]
        ldacc = singles.tile([128, NT, D], F32, tag="ldacc")
        if A:
            nc.vector.memset(ldacc, 0.0)
        # bias const for invs = exp(-lnS + ln(SCL))
        lnscl_t = singles.tile([128, 1], F32)
        nc.vector.memset(lnscl_t, NLNSCL)
        # edge constants c_j, duplicated per (d, q) pair: [128, 7, 128]
        cedge = singles.tile([128, 7, 2 * D], BT)
        nc.sync.dma_start(
            out=cedge,
            in_=bass.AP(tensor=ce_d.tensor, offset=0,
                        ap=[[0, 128], [2 * D, 7], [1, 2 * D]]),
        )
        if has_b3:
            ones1 = singles.tile([1, 128], F32)
            nc.vector.memset(ones1, 1.0)
            b3sb = singles.tile([1, ODIM], F32)
            nc.sync.dma_start(out=b3sb, in_=b3_d)
        # persistent V tiles: 6 packed f16 lanes per (slot j, dim d):
        # (e_j, ch_j, e_{j+1}, ch_{j+1}, sp_j, sp_{j+1}) = 3 f32 pairs, so the
        # walk moves left+right values of all planes in ONE f32-bitcast
        # predicated copy. Slots 0..7 (bin index k). Boundary constants
        # written once.
        NVB = int(os.environ.get("SPLINE_NVB", "2"))
        V6bufs = [singles.tile([128, RT, 8, 3, 2 * D], BT, name=f"V6b{i}")
                  for i in range(NVB)]
        for i in range(NVB):
            v = V6bufs[i].rearrange("p r s t (d q) -> p r s t d q", q=2)
            nc.gpsimd.memset(v[:, :, 0, 0, :, :], -BND)
            nc.gpsimd.memset(v[:, :, 7, 1, :, :], BND)
            nc.gpsimd.memset(v[:, :, 0, 2, :, 0], SPBOUND)
            nc.gpsimd.memset(v[:, :, 7, 2, :, 1], SPBOUND)

        x_cur, x_nxt = xa, xb

        for l in range(L):
            # ---------------- layer weights ----------------
            w1t = wpool.tile([128, HID], MM_DT, tag="w1")
            w2t = wpool.tile([128, 2, HID], MM_DT, tag="w2")
            w3t = wpool.tile([128, 2, ODIM], MM_DT, tag="w3")
            b1t = wpool.tile([128, 2], F32, tag="b1")
            b2t = wpool.tile([128, 2], F32, tag="b2")
            att = wpool.tile([64, D], MM_DT, tag="at")
            if has_lub:
                lubt = wpool.tile([128, D], F32, tag="lub")
            nc.sync.dma_start(out=w1t, in_=w1_d[l])
            nc.sync.dma_start(out=w2t, in_=w2_d[l].rearrange("k p h -> p k h"))
            nc.sync.dma_start(out=w3t, in_=w3_d[l].rearrange("k p h -> p k h"))
            nc.sync.dma_start(out=b1t, in_=b1_d[l].rearrange("t p -> p t"))
            nc.sync.dma_start(out=b2t, in_=b2_d[l].rearrange("t p -> p t"))
            nc.sync.dma_start(out=att, in_=at_d[l])
            if has_lub:
                nc.sync.dma_start(
                    out=lubt,
                    in_=bass.AP(tensor=lub_d.tensor, offset=l * D,
                                ap=[[0, 128], [1, D]]),
                )

            for chi in range(NCH):
                # ---------------- MLP chunk (transposed world) ----------------
                c0 = chi * CHW
                NHALF = max(1, CHW // 512)
                h1t = hpool.tile([128, 2, CHW], MM_DT, tag="h1")
                for m in range(2):
                    ps1 = ps_h.tile([128, CHW], F32, tag="psh")
                    for hf in range(NHALF):
                        h0 = hf * 512
                        hw_ = min(512, CHW - h0)
                        nc.tensor.matmul(
                            ps1[:, h0:h0 + hw_],
                            lhsT=w1t[:, m * 128:(m + 1) * 128],
                            rhs=ctxT[:, c0 + h0:c0 + h0 + hw_])
                    nc.scalar.activation(h1t[:, m, :], ps1, AF.Relu,
                                         bias=b1t[:, m:m + 1])
                h2t = hpool.tile([128, 2, CHW], MM_DT, tag="h2")
                for m in range(2):
                    ps2 = ps_h.tile([128, CHW], F32, tag="psh")
                    for hf in range(NHALF):
                        h0 = hf * 512
                        hw_ = min(512, CHW - h0)
                        for kk in range(2):
                            nc.tensor.matmul(
                                ps2[:, h0:h0 + hw_],
                                lhsT=w2t[:, kk, m * 128:(m + 1) * 128],
                                rhs=h1t[:, kk, h0:h0 + hw_],
                                start=(kk == 0), stop=(kk == 1))
                    nc.scalar.activation(h2t[:, m, :], ps2, AF.Relu,
                                         bias=b2t[:, m:m + 1])

                if "binning" not in A:
                    xcc_nxt = frm.tile([128, RT, D], BT, tag="xcc", name="xcc")
                    nc.gpsimd.tensor_scalar(
                        xcc_nxt,
                        x_cur[:, chi * RT_PER_CH:chi * RT_PER_CH + RT, :],
                        -BND, BND, OP.max, OP.min)
                for sub in range(RT_PER_CH // RT):
                    rt0 = chi * RT_PER_CH + sub * RT
                    V6 = V6bufs[(rt0 // RT) % NVB]
                    if "exps" not in A:
                        # exp(params) tile, one ACT write per row-tile:
                        # cols 0:1024 = uw/uh [8(k), 64(d), 2(q)] (cumsummed in
                        # place below), cols 1024:1472 = ud [7, 64].
                        EP = binp.tile([128, RT, ODIM], BT, tag="EP", name="EP")
                        E = EP[:, :, 0:1024].rearrange(
                            "p r (k d q) -> p r k d q", k=NB, d=D, q=2)
                        EU = EP[:, :, 1024:1472].rearrange(
                            "p r (j d) -> p r j d", j=7, d=D)
                    if "binning" not in A:
                        Gc6 = binp.tile([128, RT, 3, 2 * D], BT, tag="Gc6",
                                        name="Gc6")
                        xcc = xcc_nxt
                        if sub + 1 < RT_PER_CH // RT:
                            # prefetch next sub-chunk's clip ahead of this
                            # sub-chunk's formula ops in Pool's in-order queue
                            xcc_nxt = frm.tile([128, RT, D], BT, tag="xcc",
                                               name="xcc")
                            nc.gpsimd.tensor_scalar(
                                xcc_nxt, x_cur[:, rt0 + RT:rt0 + 2 * RT, :],
                                -BND, BND, OP.max, OP.min)
                    if A:
                        rq = frm.tile([128, RT], F32, tag="rq", name="rq")

                    for rti in range(RT):
                        rt = rt0 + rti
                        r0 = (sub * RT + rti) * 128
                        # -------- GEMM3: params [128 rows, 1472] --------
                        psp = ps_par.tile([128, ODIM], F32, tag="pspar")
                        nslices = [(0, 512), (512, 512), (1024, 448)]
                        for (ns, nw) in nslices:
                            for kk in range(2):
                                nc.tensor.matmul(
                                    psp[:, ns:ns + nw],
                                    lhsT=h2t[:, kk, r0:r0 + 128],
                                    rhs=w3t[:, kk, ns:ns + nw],
                                    start=(kk == 0),
                                    stop=(kk == 1) if not has_b3 else False,
                                )
                            if has_b3:
                                nc.tensor.matmul(
                                    psp[:, ns:ns + nw], lhsT=ones1,
                                    rhs=b3sb[:, ns:ns + nw],
                                    start=False, stop=True)

                        if "exps" in A:
                            nc.vector.tensor_reduce(
                                rq[:, rti:rti + 1],
                                psp[:, 0:64].unsqueeze(1),
                                mybir.AxisListType.X, OP.add)
                            continue
                        # -------- exp / softplus (ACT) --------
                        nc.scalar.activation(EP[:, rti], psp, AF.Exp)
                        # softplus(ud) = Ln(exp(ud) + 1) into BOTH sp lane
                        # destinations with one ACT op: the last AP dim strides
                        # from (slot j-1, lane 1) to (slot j, lane 0), and the
                        # input reads each value twice via a stride-0 dim.
                        vb = V6[:, rti]
                        eu = EU[:, rti]
                        sp_out = bass.AP(
                            tensor=vb.tensor,
                            offset=vb.offset + 2 * 2 * D + 1,
                            ap=[list(vb.ap[0]), [3 * 2 * D, 7], [2, D],
                                [3 * 2 * D - 1, 2]])
                        sp_in = bass.AP(
                            tensor=eu.tensor, offset=eu.offset,
                            ap=[list(eu.ap[0]), [D, 7], [1, D], [0, 2]])
                        nc.scalar.activation(sp_out, sp_in, AF.Ln, bias=1.0)

                    if "binning" in A:
                        nc.vector.tensor_reduce(
                            rq, E[:, :, 0, :, 0], mybir.AxisListType.X, OP.add)
                        if l == 0:
                            nc.vector.tensor_copy(ldacc[:, rt0:rt0 + RT, 0:1],
                                                  rq.unsqueeze(2))
                        else:
                            nc.vector.tensor_add(
                                ldacc[:, rt0:rt0 + RT, 0:1], rq.unsqueeze(2),
                                ldacc[:, rt0:rt0 + RT, 0:1])
                        continue

                    # -------- cumsum chain, in place on E (f16, DVE) --------
                    Em = E.rearrange("p r k d q -> p r k (d q)")
                    for j in range(1, NB):
                        nc.vector.tensor_add(Em[:, :, j, :], Em[:, :, j - 1, :],
                                             Em[:, :, j, :])
                    # -------- invs = SCL / S via ACT ln/exp --------
                    lnS = frm.tile([128, RT, 2 * D], F32, tag="lnS", name="lnS")
                    nc.scalar.activation(lnS, Em[:, :, NB - 1, :], AF.Ln)
                    invs = frm.tile([128, RT, 2 * D], BT, tag="invs",
                                    name="invs")
                    nc.scalar.activation(invs, lnS, AF.Exp, bias=lnscl_t,
                                         scale=-1.0)
                    # -------- edges into packed V pair 0 (f16, DVE) --------
                    V6p0 = V6[:, :, 1:8, 0, :]
                    nc.vector.tensor_mul(
                        V6p0, Em[:, :, 0:7, :],
                        invs.unsqueeze(2).to_broadcast([128, RT, 7, 2 * D]))
                    nc.vector.tensor_add(
                        V6p0, V6p0,
                        cedge.unsqueeze(1).to_broadcast([128, RT, 7, 2 * D]))
                    # right-neighbor (e, ch) duplicated into pair 1 (TC 4x)
                    nc.vector.tensor_copy(V6[:, :, 0:7, 1, :], V6p0)

                    # -------- masks (f16, 2x) + single packed walk (DVE) -----
                    su = binp.tile([128, RT, 7, D], BT, tag="su", name="su")
                    nc.vector.tensor_tensor(
                        su,
                        xcc.unsqueeze(2).to_broadcast([128, RT, 7, D]),
                        V6[:, :, 1:8, 0, :].rearrange(
                            "p r s (d q) -> p r s d q", q=2)[:, :, :, :, 0],
                        OP.is_ge)
                    V632 = V6.bitcast(F32)      # [128, RT, 8, 3, 64]
                    Gc632 = Gc6.bitcast(F32)    # [128, RT, 3, 64]
                    nc.vector.tensor_copy(
                        Gc6.rearrange("p r t m -> p r (t m)"),
                        V6[:, :, 0, :, :].rearrange("p r t m -> p r (t m)"))
                    su16 = su.bitcast(mybir.dt.uint16)
                    for j in range(1, 8):
                        nc.vector.copy_predicated(
                            Gc632,
                            su16[:, :, j - 1:j, :].to_broadcast(
                                [128, RT, 3, D]),
                            V632[:, :, j, :, :])

                    if "formula" in A:
                        nc.vector.tensor_reduce(
                            rq,
                            Gc6[:, :, 0, :].rearrange(
                                "p r (d q) -> p r d q", q=2)[:, :, :, 0],
                            mybir.AxisListType.X, OP.add)
                        if l == 0:
                            nc.vector.tensor_copy(ldacc[:, rt0:rt0 + RT, 0:1],
                                                  rq.unsqueeze(2))
                        else:
                            nc.vector.tensor_add(
                                ldacc[:, rt0:rt0 + RT, 0:1], rq.unsqueeze(2),
                                ldacc[:, rt0:rt0 + RT, 0:1])
                        continue
                    # ------------- formula (chunked, mostly f16) -------------
                    FSH = [128, RT, D]
                    G0 = Gc6[:, :, 0, :].rearrange("p r (d q) -> p r d q", q=2)
                    G1 = Gc6[:, :, 1, :].rearrange("p r (d q) -> p r d q", q=2)
                    le, lch = G0[:, :, :, 0], G0[:, :, :, 1]
                    re_, rch = G1[:, :, :, 0], G1[:, :, :, 1]
                    Gdp = frm.tile([128, RT, 2 * D], BT, tag="Gdp", name="Gdp")
                    nc.vector.tensor_scalar(Gdp, Gc6[:, :, 2, :],
                                            MIN_D, None, OP.add)
                    Gdpq = Gdp.rearrange("p r (d q) -> p r d q", q=2)
                    ind, ind1 = Gdpq[:, :, :, 0], Gdpq[:, :, :, 1]

                    def ft(tag, dt_=BT):
                        return frm.tile(FSH, dt_, tag=tag, name=tag)

                    xt = x_cur[:, rt0:rt0 + RT, :]
                    # 1/in_w and 1/denom via ACT ln/exp keeps the mults f16-2x
                    in_w = ft("in_w"); nc.gpsimd.tensor_sub(in_w, re_, le)
                    lnw = ft("lnw")
                    nc.scalar.activation(lnw, in_w, AF.Ln)
                    rw = ft("rw")
                    nc.scalar.activation(rw, lnw, AF.Exp, scale=-1.0)
                    tnum = ft("tnum"); nc.gpsimd.tensor_sub(tnum, xcc, le)
                    th = ft("th"); nc.vector.tensor_mul(th, tnum, rw)
                    in_h = ft("in_h"); nc.gpsimd.tensor_sub(in_h, rch, lch)
                    idel = ft("idel"); nc.vector.tensor_mul(idel, in_h, rw)
                    # squares on ACT; (1-th)^2 fused via scale/bias
                    th2 = ft("th2"); nc.scalar.activation(th2, th, AF.Square)
                    omt2 = ft("omt2")
                    nc.scalar.activation(omt2, th, AF.Square, bias=1.0, scale=-1.0)
                    idel2 = ft("idel2")
                    nc.scalar.activation(idel2, idel, AF.Square)
                    tomt = ft("tomt"); nc.vector.tensor_sub(tomt, th, th2)
                    t1 = ft("t1"); nc.vector.tensor_mul(t1, idel, th2)
                    t2 = ft("t2"); nc.vector.tensor_mul(t2, ind, tomt)
                    nsum = t1; nc.vector.tensor_add(nsum, t1, t2)
                    numer = in_h; nc.vector.tensor_mul(numer, in_h, nsum)
                    dd = ft("dd"); nc.gpsimd.tensor_add(dd, ind, ind1)
                    dd2 = ft("dd2")
                    nc.vector.scalar_tensor_tensor(dd2, idel, -2.0, dd,
                                                   OP.mult, OP.add)
                    dt = dd2; nc.vector.tensor_mul(dt, dd2, tomt)
                    denom = ft("denom")
                    nc.gpsimd.tensor_add(denom, idel, dt)
                    lnden = ft("lnden")
                    nc.scalar.activation(lnden, denom, AF.Ln)
                    rden = ft("rden")
                    nc.scalar.activation(rden, lnden, AF.Exp, scale=-1.0)
                    yq = rden; nc.vector.tensor_mul(yq, numer, rden)
                    y = ft("y", F32); nc.gpsimd.tensor_add(y, lch, yq)
                    u1 = ft("u1"); nc.vector.tensor_mul(u1, th2, ind1)
                    idt = ft("idt"); nc.vector.tensor_mul(idt, idel, tomt)
                    u2 = u1
                    nc.vector.scalar_tensor_tensor(u2, idt, 2.0, u1,
                                                   OP.mult, OP.add)
                    u3 = ft("u3"); nc.gpsimd.tensor_mul(u3, ind, omt2)
                    uu = u2; nc.vector.tensor_add(uu, u2, u3)
                    dnum = idel2; nc.vector.tensor_mul(dnum, uu, idel2)
                    lnd = ft("lnd"); nc.scalar.activation(lnd, dnum, AF.Ln)
                    ldt = ft("ldt")
                    nc.vector.scalar_tensor_tensor(ldt, lnden, -2.0, lnd,
                                                   OP.mult, OP.add)
                    # inside mask + select (f16 mask, bitcast for predication)
                    absx = ft("absx")
                    nc.scalar.activation(absx, xt, AF.Abs)
                    insu = frm.tile([128, RT, D], BT, tag="insu", name="insu")
                    nc.vector.tensor_scalar(insu, absx, BND, None, OP.is_le)
                    yfin = ft("yfin", F32)
                    nc.gpsimd.tensor_copy(yfin, xt)
                    nc.vector.copy_predicated(yfin, insu.bitcast(mybir.dt.uint16),
                                              y)
                    # masked per-dim logdet accumulate, chained across layers
                    ldm = ft("ldm")
                    nc.vector.tensor_mul(ldm, ldt, insu)
                    if l == 0:
                        nc.gpsimd.tensor_copy(ldacc[:, rt0:rt0 + RT, :], ldm)
                    else:
                        nc.gpsimd.tensor_add(ldacc[:, rt0:rt0 + RT, :], ldm,
                                             ldacc[:, rt0:rt0 + RT, :])

                    # -------- LU per row-tile: x' = y @ A^T + b --------
                    if "lu" in A:
                        continue
                    for rti in range(RT):
                        rt = rt0 + rti
                        pst = ps_lu.tile([64, 128], F32, tag="pst")
                        nc.tensor.transpose(pst, yfin[:, rti, :], ident)
                        yT = frm.tile([64, 128], MM_DT, tag="yT", name="yT")
                        nc.scalar.copy(yT, pst)
                        psx = ps_lu.tile([128, D], F32, tag="psx")
                        nc.tensor.matmul(psx, lhsT=yT, rhs=att)
                        if has_lub:
                            nc.vector.tensor_add(x_nxt[:, rt, :], psx, lubt)
                        else:
                            nc.scalar.copy(x_nxt[:, rt, :], psx)


            if "lu" not in A:
                x_cur, x_nxt = x_nxt, x_cur

        # ---------------- final: out = -0.5*sum(x^2) + sum_d ld + const -----
        xsq = singles.tile([128, NT, D], F32)
        nc.vector.tensor_mul(xsq, x_cur, x_cur)
        ov2 = singles.tile([128, NT, D], F32)
        nc.vector.scalar_tensor_tensor(ov2, xsq, -0.5, ldacc, OP.mult, OP.add)
        ov = singles.tile([128, NT], F32)
        nc.vector.tensor_reduce(ov, ov2, mybir.AxisListType.X, OP.add)
        # const added on host (exact); DMA out
        nc.sync.dma_start(out=out_d.rearrange("(t p) -> p t", p=128), in_=ov)

    nc.compile()
    return nc


# ------------------------- host side -------------------------

def _host_prep(inputs):
    x = np.ascontiguousarray(inputs["inputs"].astype(np.float32))
    ctx = inputs["context"].astype(np.float32)
    W1 = inputs["W1"].astype(np.float32)
    W2 = inputs["W2"].astype(np.float32)
    W3 = inputs["W3"].astype(np.float32)
    b1 = inputs["b1"].astype(np.float32)
    b2 = inputs["b2"].astype(np.float32)
    b3 = inputs["b3"].astype(np.float32)

    mmnp = np.float16 if MM_DT == F16 else np.float32

    cols = np.arange(D * 23).reshape(D, 23)
    # uw/uh interleaved (k, d, q): the (cumw, cumh) pair for one dim lands in
    # adjacent f16 elements (walked as one f32); ud stays (j, d).
    wh = np.stack([cols[:, 0:8], cols[:, 8:16]], axis=-1)  # [d, k, 2]
    perm = np.concatenate([
        wh.transpose(1, 0, 2).reshape(-1),  # (k, d, q)
        cols[:, 16:23].T.reshape(-1),
    ])
    W3r = W3[:, :, perm]
    b3r = b3[:, perm]
    has_b3 = bool(np.any(b3r != 0.0))

    ctxT = np.ascontiguousarray(ctx.T.astype(mmnp))                 # [128, B]
    w1 = np.ascontiguousarray(W1.astype(mmnp))                      # [L,128,256]
    w2 = np.ascontiguousarray(
        W2.reshape(L, 2, 128, HID).astype(mmnp))                    # [L,2,128,256]
    w3 = np.ascontiguousarray(
        W3r.reshape(L, 2, 128, ODIM).astype(mmnp))                  # [L,2,128,1472]
    bias1 = np.ascontiguousarray(b1.reshape(L, 2, 128))
    bias2 = np.ascontiguousarray(b2.reshape(L, 2, 128))

    li = np.tril_indices(D, -1)
    ui = np.triu_indices(D, 1)
    at = np.zeros((L, D, D), np.float32)
    lld = 0.0
    for l in range(L):
        Lm = np.eye(D, dtype=np.float64)
        Lm[li] = inputs["lower_entries"][l].astype(np.float64)
        diag = np.log1p(np.exp(inputs["upper_diag"][l].astype(np.float64))) + 1e-3
        Um = np.zeros((D, D), np.float64)
        Um[ui] = inputs["upper_entries"][l].astype(np.float64)
        Um += np.diag(diag)
        at[l] = (Lm @ Um).T.astype(np.float32)  # cast to mm dtype below
        lld += float(np.sum(np.log(diag)))
    lub = np.ascontiguousarray(inputs["lu_bias"].astype(np.float32))

    cj = ((2.0 * BND * MIN_W) * np.arange(1, 8).astype(np.float32) - BND)
    btnp = np.float16 if BT == F16 else np.float32
    cedge = np.ascontiguousarray(
        np.broadcast_to(cj[:, None], (7, 2 * D)).astype(btnp))

    const_total = -0.5 * D * float(np.log(2.0 * np.pi)) + lld

    has_lub = bool(np.any(lub != 0.0))
    shared = dict(w1=w1, w2=w2, w3=w3, bias1=bias1, bias2=bias2,
                  at=at.astype(mmnp), cedge=cedge)
    if has_lub:
        shared["lub"] = lub
    if has_b3:
        shared["bias3"] = np.ascontiguousarray(b3r.reshape(1, ODIM))
    return x, ctxT, shared, has_b3, has_lub, const_total


_CACHE = {}


def kernel(**inputs):
    rpc = inputs["inputs"].shape[0] // N_CORES
    x, ctxT, shared, has_b3, has_lub, const_total = _host_prep(inputs)

    key = (rpc, has_b3, has_lub)
    if key not in _CACHE:
        _CACHE[key] = build_program(rpc, has_b3=has_b3, has_lub=has_lub)
    nc = _CACHE[key]

    in_maps = []
    for c in range(N_CORES):
        m = dict(shared)
        m["xin"] = np.ascontiguousarray(x[c * rpc:(c + 1) * rpc])
        m["ctxT"] = np.ascontiguousarray(ctxT[:, c * rpc:(c + 1) * rpc])
        in_maps.append(m)

    res = bass_utils.run_bass_kernel_spmd(nc, in_maps, core_ids=list(range(N_CORES)))
    out = np.concatenate([r["out"] for r in res.results])
    return (out + np.float32(const_total)).astype(np.float32)


# revision 82
# speedup vs baseline: 1.0142x; 1.0053x over previous
"""Trainium2 Bass kernel for nn_ConditionalSplineFlow (8-core data parallel).

Layout strategy:
  - MLP runs in "transposed world": activations [feature, rows] so weight
    matrices act as lhsT directly; final GEMM flips orientation using h2^T as
    lhsT, giving params [rows(part), 1472(free)] with W3 columns reordered to
    [uw(k-major,d-inner) | uh | ud(j-major)].
  - Bin space is batched per sub-chunk of RT row-tiles with uw/uh columns
    interleaved (k, d, q) so the (cumw, cumh) edge pair for one dim sits in
    adjacent f16 elements: the bin-search walk moves both via one f32-bitcast
    predicated copy (half the element count). Cumsum chain is an unrolled f16
    DVE tensor-add chain (2x perf mode), 1/S comes from ACT ln/exp
    (exp(-lnS + ln(SCL)) = SCL/S), edges are two batched f16 DVE tensor ops.
  - Spline formula is elementwise in (row, dim), chunked per sub-chunk.
  - LU layer folded to  x' = y @ (L@U)^T + b  via PE transpose + matmul.
  - Per-layer logdet of LU and the gaussian constant are folded on host.
"""
import os
import numpy as np
from contextlib import ExitStack

import concourse.bass as bass
import concourse.bacc as bacc
import concourse.tile as tile
import concourse.mybir as mybir
from concourse import bass_utils
from concourse.masks import make_identity

# Pin all activations to the one table set that covers Exp/Ln/Relu/Copy/Abs —
# the default per-function chooser ping-pongs between sets (~2.6us per swap,
# once per row-tile). Masking the other sets (order preserved, so positional
# set ids stay valid) forces a single resident table.
_PINNED_ACT_SET = "natural_log_exp_and_others"
_orig_gat = bacc.get_activation_tables


def _gat_pinned(arch):
    tabs = _orig_gat(arch)
    return {name: (fns if name == _PINNED_ACT_SET else set())
            for name, fns in tabs.items()}


bacc.get_activation_tables = _gat_pinned

F32 = mybir.dt.float32
F16 = mybir.dt.float16
U8 = mybir.dt.uint8
AF = mybir.ActivationFunctionType
OP = mybir.AluOpType

D = 64
NB = 8
L = 5
HID = 256
ODIM = 1472
BND = 5.0
MIN_W = 1e-3
MIN_D = 1e-3
SCL = 2.0 * BND * (1.0 - MIN_W * NB)      # 9.92
SPBOUND = 1.0 - MIN_D                      # softplus value at padded boundary
N_CORES = 8
BATCH = 32768

MM_DT = F16   # matmul dtype for MLP
BT = F16      # bin-space dtype (V / Gc / chain / edges)


def build_program(rpc, has_b3=False, has_lub=True, ablate=()):
    """Build the single-core program for `rpc` rows. Returns nc.

    `ablate` (dev-only; never set by kernel()): drop pipeline sections to
    isolate their HW cost. Subsets of {"exps", "binning", "formula", "lu"}.
    Later sections imply earlier ones stay: ablating "exps" also ablates
    "binning"/"formula"/"lu" (they consume its outputs)."""
    A = set(ablate)
    if "exps" in A:
        A |= {"binning"}
    if "binning" in A:
        A |= {"formula"}
    if "formula" in A:
        A |= {"lu"}
    nc = bacc.Bacc(
        "TRN2", target_bir_lowering=False, debug=False,
        enable_asserts=False, num_devices=N_CORES,
    )
    NT = rpc // 128               # row tiles
    CHW = min(rpc, int(os.environ.get("SPLINE_CHW", "1024")))  # mlp chunk rows
    NCH = rpc // CHW              # chunks
    RT_PER_CH = CHW // 128
    RT = int(os.environ.get("SPLINE_RT", "4"))   # bin sub-chunk (row tiles)
    assert RT_PER_CH % RT == 0
    NLNSCL = float(np.log(SCL))

    # ---------------- DRAM I/O ----------------
    xin_d = nc.dram_tensor("xin", [rpc, D], F32, kind="ExternalInput").ap()
    ctxT_d = nc.dram_tensor("ctxT", [128, rpc], MM_DT, kind="ExternalInput").ap()
    w1_d = nc.dram_tensor("w1", [L, 128, HID], MM_DT, kind="ExternalInput").ap()
    w2_d = nc.dram_tensor("w2", [L, 2, 128, HID], MM_DT, kind="ExternalInput").ap()
    w3_d = nc.dram_tensor("w3", [L, 2, 128, ODIM], MM_DT, kind="ExternalInput").ap()
    b1_d = nc.dram_tensor("bias1", [L, 2, 128], F32, kind="ExternalInput").ap()
    b2_d = nc.dram_tensor("bias2", [L, 2, 128], F32, kind="ExternalInput").ap()
    at_d = nc.dram_tensor("at", [L, D, D], MM_DT, kind="ExternalInput").ap()
    if has_lub:
        lub_d = nc.dram_tensor("lub", [L, D], F32, kind="ExternalInput").ap()
    ce_d = nc.dram_tensor("cedge", [7, 2 * D], BT, kind="ExternalInput").ap()
    if has_b3:
        b3_d = nc.dram_tensor("bias3", [1, ODIM], F32, kind="ExternalInput").ap()
    out_d = nc.dram_tensor("out", [rpc], F32, kind="ExternalOutput").ap()

    with tile.TileContext(nc) as tc, ExitStack() as ctx:
        # ---------------- pools ----------------
        singles = ctx.enter_context(tc.tile_pool(name="singles", bufs=1))
        wpool = ctx.enter_context(tc.tile_pool(name="wpool", bufs=2))
        hpool = ctx.enter_context(tc.tile_pool(name="hpool", bufs=2))
        binp = ctx.enter_context(tc.tile_pool(name="binp", bufs=2))
        frm = ctx.enter_context(tc.tile_pool(
            name="frm", bufs=int(os.environ.get("SPLINE_FRM_BUFS", "2"))))
        ps_par = ctx.enter_context(tc.tile_pool(
            name="ps_par", bufs=int(os.environ.get("SPLINE_PSPAR_BUFS", "1")),
            space="PSUM"))
        ps_h = ctx.enter_context(tc.tile_pool(
            name="ps_h", bufs=int(os.environ.get("SPLINE_PSH_BUFS", "1")),
            space="PSUM"))
        ps_lu = ctx.enter_context(tc.tile_pool(
            name="ps_lu", bufs=int(os.environ.get("SPLINE_PSLU_BUFS", "1")),
            space="PSUM"))

        # ---------------- resident tiles ----------------
        ident = singles.tile([128, 128], F32)
        make_identity(nc, ident)
        ctxT = singles.tile([128, rpc], MM_DT)
        nc.sync.dma_start(out=ctxT, in_=ctxT_d)
        # PE p-state warmup: ~3us of back-to-back dummy matmuls during the
        # initial DMA window so the first real GEMMs run at full clock
        NWARM = int(os.environ.get("SPLINE_WARMUP_MM", "24"))
        if NWARM:
            ps_warm = ps_h.tile([128, CHW], F32, tag="psh")
            for _ in range(NWARM):
                nc.tensor.matmul(ps_warm[:, 0:128], lhsT=ident, rhs=ident)
        # x ping-pong, resident across a layer
        xa = singles.tile([128, NT, D], F32, tag="xa")
        xb = singles.tile([128, NT, D], F32, tag="xb")
        nc.sync.dma_start(out=xa, in_=xin_d.rearrange("(t p) d -> p t d", p=128))
        # per-dim logdet accumulator [128, NT, D]
        ldacc = singles.tile([128, NT, D], F32, tag="ldacc")
        if A:
            nc.vector.memset(ldacc, 0.0)
        # bias const for invs = exp(-lnS + ln(SCL))
        lnscl_t = singles.tile([128, 1], F32)
        nc.vector.memset(lnscl_t, NLNSCL)
        # edge constants c_j, duplicated per (d, q) pair: [128, 7, 128]
        cedge = singles.tile([128, 7, 2 * D], BT)
        nc.sync.dma_start(
            out=cedge,
            in_=bass.AP(tensor=ce_d.tensor, offset=0,
                        ap=[[0, 128], [2 * D, 7], [1, 2 * D]]),
        )
        if has_b3:
            ones1 = singles.tile([1, 128], F32)
            nc.vector.memset(ones1, 1.0)
            b3sb = singles.tile([1, ODIM], F32)
            nc.sync.dma_start(out=b3sb, in_=b3_d)
        # persistent V tiles: 6 packed f16 lanes per (slot j, dim d):
        # (e_j, ch_j, e_{j+1}, ch_{j+1}, sp_j, sp_{j+1}) = 3 f32 pairs, so the
        # walk moves left+right values of all planes in ONE f32-bitcast
        # predicated copy. Slots 0..7 (bin index k). Boundary constants
        # written once.
        NVB = int(os.environ.get("SPLINE_NVB", "2"))
        V6bufs = [singles.tile([128, RT, 8, 3, 2 * D], BT, name=f"V6b{i}")
                  for i in range(NVB)]
        for i in range(NVB):
            v = V6bufs[i].rearrange("p r s t (d q) -> p r s t d q", q=2)
            nc.gpsimd.memset(v[:, :, 0, 0, :, :], -BND)
            nc.gpsimd.memset(v[:, :, 7, 1, :, :], BND)
            nc.gpsimd.memset(v[:, :, 0, 2, :, 0], SPBOUND)
            nc.gpsimd.memset(v[:, :, 7, 2, :, 1], SPBOUND)

        x_cur, x_nxt = xa, xb

        for l in range(L):
            # ---------------- layer weights ----------------
            w1t = wpool.tile([128, HID], MM_DT, tag="w1")
            w2t = wpool.tile([128, 2, HID], MM_DT, tag="w2")
            w3t = wpool.tile([128, 2, ODIM], MM_DT, tag="w3")
            b1t = wpool.tile([128, 2], F32, tag="b1")
            b2t = wpool.tile([128, 2], F32, tag="b2")
            att = wpool.tile([64, D], MM_DT, tag="at")
            if has_lub:
                lubt = wpool.tile([128, D], F32, tag="lub")
            nc.sync.dma_start(out=w1t, in_=w1_d[l])
            nc.sync.dma_start(out=w2t, in_=w2_d[l].rearrange("k p h -> p k h"))
            nc.sync.dma_start(out=w3t, in_=w3_d[l].rearrange("k p h -> p k h"))
            nc.sync.dma_start(out=b1t, in_=b1_d[l].rearrange("t p -> p t"))
            nc.sync.dma_start(out=b2t, in_=b2_d[l].rearrange("t p -> p t"))
            nc.sync.dma_start(out=att, in_=at_d[l])
            if has_lub:
                nc.sync.dma_start(
                    out=lubt,
                    in_=bass.AP(tensor=lub_d.tensor, offset=l * D,
                                ap=[[0, 128], [1, D]]),
                )

            for chi in range(NCH):
                # ---------------- MLP chunk (transposed world) ----------------
                c0 = chi * CHW
                NHALF = max(1, CHW // 512)
                h1t = hpool.tile([128, 2, CHW], MM_DT, tag="h1")
                for m in range(2):
                    ps1 = ps_h.tile([128, CHW], F32, tag="psh")
                    for hf in range(NHALF):
                        h0 = hf * 512
                        hw_ = min(512, CHW - h0)
                        nc.tensor.matmul(
                            ps1[:, h0:h0 + hw_],
                            lhsT=w1t[:, m * 128:(m + 1) * 128],
                            rhs=ctxT[:, c0 + h0:c0 + h0 + hw_])
                    nc.scalar.activation(h1t[:, m, :], ps1, AF.Relu,
                                         bias=b1t[:, m:m + 1])
                h2t = hpool.tile([128, 2, CHW], MM_DT, tag="h2")
                for m in range(2):
                    ps2 = ps_h.tile([128, CHW], F32, tag="psh")
                    for hf in range(NHALF):
                        h0 = hf * 512
                        hw_ = min(512, CHW - h0)
                        for kk in range(2):
                            nc.tensor.matmul(
                                ps2[:, h0:h0 + hw_],
                                lhsT=w2t[:, kk, m * 128:(m + 1) * 128],
                                rhs=h1t[:, kk, h0:h0 + hw_],
                                start=(kk == 0), stop=(kk == 1))
                    nc.scalar.activation(h2t[:, m, :], ps2, AF.Relu,
                                         bias=b2t[:, m:m + 1])

                if "binning" not in A:
                    xcc_nxt = frm.tile([128, RT, D], BT, tag="xcc", name="xcc")
                    nc.gpsimd.tensor_scalar(
                        xcc_nxt,
                        x_cur[:, chi * RT_PER_CH:chi * RT_PER_CH + RT, :],
                        -BND, BND, OP.max, OP.min)
                for sub in range(RT_PER_CH // RT):
                    rt0 = chi * RT_PER_CH + sub * RT
                    V6 = V6bufs[(rt0 // RT) % NVB]
                    if "exps" not in A:
                        # exp(params) tile, one ACT write per row-tile:
                        # cols 0:1024 = uw/uh [8(k), 64(d), 2(q)] (cumsummed in
                        # place below), cols 1024:1472 = ud [7, 64].
                        EP = binp.tile([128, RT, ODIM], BT, tag="EP", name="EP")
                        E = EP[:, :, 0:1024].rearrange(
                            "p r (k d q) -> p r k d q", k=NB, d=D, q=2)
                        EU = EP[:, :, 1024:1472].rearrange(
                            "p r (j d) -> p r j d", j=7, d=D)
                    if "binning" not in A:
                        Gc6 = binp.tile([128, RT, 3, 2 * D], BT, tag="Gc6",
                                        name="Gc6")
                        xcc = xcc_nxt
                        if sub + 1 < RT_PER_CH // RT:
                            # prefetch next sub-chunk's clip ahead of this
                            # sub-chunk's formula ops in Pool's in-order queue
                            xcc_nxt = frm.tile([128, RT, D], BT, tag="xcc",
                                               name="xcc")
                            nc.gpsimd.tensor_scalar(
                                xcc_nxt, x_cur[:, rt0 + RT:rt0 + 2 * RT, :],
                                -BND, BND, OP.max, OP.min)
                    if A:
                        rq = frm.tile([128, RT], F32, tag="rq", name="rq")

                    for rti in range(RT):
                        rt = rt0 + rti
                        r0 = (sub * RT + rti) * 128
                        # -------- GEMM3: params [128 rows, 1472] --------
                        psp = ps_par.tile([128, ODIM], F32, tag="pspar")
                        nslices = [(0, 512), (512, 512), (1024, 448)]
                        for (ns, nw) in nslices:
                            for kk in range(2):
                                nc.tensor.matmul(
                                    psp[:, ns:ns + nw],
                                    lhsT=h2t[:, kk, r0:r0 + 128],
                                    rhs=w3t[:, kk, ns:ns + nw],
                                    start=(kk == 0),
                                    stop=(kk == 1) if not has_b3 else False,
                                )
                            if has_b3:
                                nc.tensor.matmul(
                                    psp[:, ns:ns + nw], lhsT=ones1,
                                    rhs=b3sb[:, ns:ns + nw],
                                    start=False, stop=True)

                        if "exps" in A:
                            nc.vector.tensor_reduce(
                                rq[:, rti:rti + 1],
                                psp[:, 0:64].unsqueeze(1),
                                mybir.AxisListType.X, OP.add)
                            continue
                        # -------- exp / softplus (ACT) --------
                        nc.scalar.activation(EP[:, rti], psp, AF.Exp)
                        # softplus(ud) = Ln(exp(ud) + 1) into BOTH sp lane
                        # destinations with one ACT op: the last AP dim strides
                        # from (slot j-1, lane 1) to (slot j, lane 0), and the
                        # input reads each value twice via a stride-0 dim.
                        vb = V6[:, rti]
                        eu = EU[:, rti]
                        sp_out = bass.AP(
                            tensor=vb.tensor,
                            offset=vb.offset + 2 * 2 * D + 1,
                            ap=[list(vb.ap[0]), [3 * 2 * D, 7], [2, D],
                                [3 * 2 * D - 1, 2]])
                        sp_in = bass.AP(
                            tensor=eu.tensor, offset=eu.offset,
                            ap=[list(eu.ap[0]), [D, 7], [1, D], [0, 2]])
                        nc.scalar.activation(sp_out, sp_in, AF.Ln, bias=1.0)

                    if "binning" in A:
                        nc.vector.tensor_reduce(
                            rq, E[:, :, 0, :, 0], mybir.AxisListType.X, OP.add)
                        if l == 0:
                            nc.vector.tensor_copy(ldacc[:, rt0:rt0 + RT, 0:1],
                                                  rq.unsqueeze(2))
                        else:
                            nc.vector.tensor_add(
                                ldacc[:, rt0:rt0 + RT, 0:1], rq.unsqueeze(2),
                                ldacc[:, rt0:rt0 + RT, 0:1])
                        continue

                    # -------- cumsum chain, in place on E (f16, DVE) --------
                    Em = E.rearrange("p r k d q -> p r k (d q)")
                    for j in range(1, NB):
                        nc.vector.tensor_add(Em[:, :, j, :], Em[:, :, j - 1, :],
                                             Em[:, :, j, :])
                    # -------- invs = SCL / S via ACT ln/exp --------
                    lnS = frm.tile([128, RT, 2 * D], F32, tag="lnS", name="lnS")
                    nc.scalar.activation(lnS, Em[:, :, NB - 1, :], AF.Ln)
                    invs = frm.tile([128, RT, 2 * D], BT, tag="invs",
                                    name="invs")
                    nc.scalar.activation(invs, lnS, AF.Exp, bias=lnscl_t,
                                         scale=-1.0)
                    # -------- edges into packed V pair 0 (f16, DVE) --------
                    V6p0 = V6[:, :, 1:8, 0, :]
                    nc.vector.tensor_mul(
                        V6p0, Em[:, :, 0:7, :],
                        invs.unsqueeze(2).to_broadcast([128, RT, 7, 2 * D]))
                    nc.vector.tensor_add(
                        V6p0, V6p0,
                        cedge.unsqueeze(1).to_broadcast([128, RT, 7, 2 * D]))
                    # right-neighbor (e, ch) duplicated into pair 1 (TC 4x)
                    nc.vector.tensor_copy(V6[:, :, 0:7, 1, :], V6p0)

                    # -------- masks (f16, 2x) + single packed walk (DVE) -----
                    su = binp.tile([128, RT, 7, D], BT, tag="su", name="su")
                    nc.vector.tensor_tensor(
                        su,
                        xcc.unsqueeze(2).to_broadcast([128, RT, 7, D]),
                        V6[:, :, 1:8, 0, :].rearrange(
                            "p r s (d q) -> p r s d q", q=2)[:, :, :, :, 0],
                        OP.is_ge)
                    V632 = V6.bitcast(F32)      # [128, RT, 8, 3, 64]
                    Gc632 = Gc6.bitcast(F32)    # [128, RT, 3, 64]
                    nc.vector.tensor_copy(
                        Gc6.rearrange("p r t m -> p r (t m)"),
                        V6[:, :, 0, :, :].rearrange("p r t m -> p r (t m)"))
                    su16 = su.bitcast(mybir.dt.uint16)
                    for j in range(1, 8):
                        nc.vector.copy_predicated(
                            Gc632,
                            su16[:, :, j - 1:j, :].to_broadcast(
                                [128, RT, 3, D]),
                            V632[:, :, j, :, :])

                    if "formula" in A:
                        nc.vector.tensor_reduce(
                            rq,
                            Gc6[:, :, 0, :].rearrange(
                                "p r (d q) -> p r d q", q=2)[:, :, :, 0],
                            mybir.AxisListType.X, OP.add)
                        if l == 0:
                            nc.vector.tensor_copy(ldacc[:, rt0:rt0 + RT, 0:1],
                                                  rq.unsqueeze(2))
                        else:
                            nc.vector.tensor_add(
                                ldacc[:, rt0:rt0 + RT, 0:1], rq.unsqueeze(2),
                                ldacc[:, rt0:rt0 + RT, 0:1])
                        continue
                    # ------------- formula (chunked, mostly f16) -------------
                    FSH = [128, RT, D]
                    G0 = Gc6[:, :, 0, :].rearrange("p r (d q) -> p r d q", q=2)
                    G1 = Gc6[:, :, 1, :].rearrange("p r (d q) -> p r d q", q=2)
                    le, lch = G0[:, :, :, 0], G0[:, :, :, 1]
                    re_, rch = G1[:, :, :, 0], G1[:, :, :, 1]
                    Gdp = frm.tile([128, RT, 2 * D], BT, tag="Gdp", name="Gdp")
                    nc.vector.tensor_scalar(Gdp, Gc6[:, :, 2, :],
                                            MIN_D, None, OP.add)
                    Gdpq = Gdp.rearrange("p r (d q) -> p r d q", q=2)
                    ind, ind1 = Gdpq[:, :, :, 0], Gdpq[:, :, :, 1]

                    def ft(tag, dt_=BT):
                        return frm.tile(FSH, dt_, tag=tag, name=tag)

                    xt = x_cur[:, rt0:rt0 + RT, :]
                    # 1/in_w and 1/denom via ACT ln/exp keeps the mults f16-2x
                    in_w = ft("in_w"); nc.gpsimd.tensor_sub(in_w, re_, le)
                    lnw = ft("lnw")
                    nc.scalar.activation(lnw, in_w, AF.Ln)
                    rw = ft("rw")
                    nc.scalar.activation(rw, lnw, AF.Exp, scale=-1.0)
                    tnum = ft("tnum"); nc.gpsimd.tensor_sub(tnum, xcc, le)
                    th = ft("th"); nc.vector.tensor_mul(th, tnum, rw)
                    in_h = ft("in_h"); nc.gpsimd.tensor_sub(in_h, rch, lch)
                    idel = ft("idel"); nc.vector.tensor_mul(idel, in_h, rw)
                    # squares on ACT; (1-th)^2 fused via scale/bias
                    th2 = ft("th2"); nc.scalar.activation(th2, th, AF.Square)
                    omt2 = ft("omt2")
                    nc.scalar.activation(omt2, th, AF.Square, bias=1.0, scale=-1.0)
                    idel2 = ft("idel2")
                    nc.scalar.activation(idel2, idel, AF.Square)
                    tomt = ft("tomt"); nc.vector.tensor_sub(tomt, th, th2)
                    t1 = ft("t1"); nc.vector.tensor_mul(t1, idel, th2)
                    t2 = ft("t2"); nc.vector.tensor_mul(t2, ind, tomt)
                    nsum = t1; nc.vector.tensor_add(nsum, t1, t2)
                    numer = in_h; nc.vector.tensor_mul(numer, in_h, nsum)
                    dd = ft("dd"); nc.vector.tensor_add(dd, ind, ind1)
                    dd2 = ft("dd2")
                    nc.vector.scalar_tensor_tensor(dd2, idel, -2.0, dd,
                                                   OP.mult, OP.add)
                    dt = dd2; nc.vector.tensor_mul(dt, dd2, tomt)
                    denom = ft("denom")
                    nc.gpsimd.tensor_add(denom, idel, dt)
                    lnden = ft("lnden")
                    nc.scalar.activation(lnden, denom, AF.Ln)
                    rden = ft("rden")
                    nc.scalar.activation(rden, lnden, AF.Exp, scale=-1.0)
                    yq = rden; nc.vector.tensor_mul(yq, numer, rden)
                    y = ft("y", F32); nc.gpsimd.tensor_add(y, lch, yq)
                    u1 = ft("u1"); nc.vector.tensor_mul(u1, th2, ind1)
                    idt = ft("idt"); nc.vector.tensor_mul(idt, idel, tomt)
                    u2 = u1
                    nc.vector.scalar_tensor_tensor(u2, idt, 2.0, u1,
                                                   OP.mult, OP.add)
                    u3 = ft("u3"); nc.gpsimd.tensor_mul(u3, ind, omt2)
                    uu = u2; nc.vector.tensor_add(uu, u2, u3)
                    dnum = idel2; nc.vector.tensor_mul(dnum, uu, idel2)
                    lnd = ft("lnd"); nc.scalar.activation(lnd, dnum, AF.Ln)
                    ldt = ft("ldt")
                    nc.vector.scalar_tensor_tensor(ldt, lnden, -2.0, lnd,
                                                   OP.mult, OP.add)
                    # inside mask + select (f16 mask, bitcast for predication)
                    absx = ft("absx")
                    nc.scalar.activation(absx, xt, AF.Abs)
                    insu = frm.tile([128, RT, D], BT, tag="insu", name="insu")
                    nc.vector.tensor_scalar(insu, absx, BND, None, OP.is_le)
                    yfin = ft("yfin", F32)
                    nc.gpsimd.tensor_copy(yfin, xt)
                    nc.vector.copy_predicated(yfin, insu.bitcast(mybir.dt.uint16),
                                              y)
                    # masked per-dim logdet accumulate, chained across layers
                    ldm = ft("ldm")
                    nc.vector.tensor_mul(ldm, ldt, insu)
                    if l == 0:
                        nc.gpsimd.tensor_copy(ldacc[:, rt0:rt0 + RT, :], ldm)
                    else:
                        nc.gpsimd.tensor_add(ldacc[:, rt0:rt0 + RT, :], ldm,
                                             ldacc[:, rt0:rt0 + RT, :])

                    # -------- LU per row-tile: x' = y @ A^T + b --------
                    if "lu" in A:
                        continue
                    for rti in range(RT):
                        rt = rt0 + rti
                        pst = ps_lu.tile([64, 128], F32, tag="pst")
                        nc.tensor.transpose(pst, yfin[:, rti, :], ident)
                        yT = frm.tile([64, 128], MM_DT, tag="yT", name="yT")
                        nc.scalar.copy(yT, pst)
                        psx = ps_lu.tile([128, D], F32, tag="psx")
                        nc.tensor.matmul(psx, lhsT=yT, rhs=att)
                        if has_lub:
                            nc.vector.tensor_add(x_nxt[:, rt, :], psx, lubt)
                        else:
                            nc.scalar.copy(x_nxt[:, rt, :], psx)


            if "lu" not in A:
                x_cur, x_nxt = x_nxt, x_cur

        # ---------------- final: out = -0.5*sum(x^2) + sum_d ld + const -----
        xsq = singles.tile([128, NT, D], F32)
        nc.vector.tensor_mul(xsq, x_cur, x_cur)
        ov2 = singles.tile([128, NT, D], F32)
        nc.vector.scalar_tensor_tensor(ov2, xsq, -0.5, ldacc, OP.mult, OP.add)
        ov = singles.tile([128, NT], F32)
        nc.vector.tensor_reduce(ov, ov2, mybir.AxisListType.X, OP.add)
        # const added on host (exact); DMA out
        nc.sync.dma_start(out=out_d.rearrange("(t p) -> p t", p=128), in_=ov)

    nc.compile()
    return nc


# ------------------------- host side -------------------------

def _host_prep(inputs):
    x = np.ascontiguousarray(inputs["inputs"].astype(np.float32))
    ctx = inputs["context"].astype(np.float32)
    W1 = inputs["W1"].astype(np.float32)
    W2 = inputs["W2"].astype(np.float32)
    W3 = inputs["W3"].astype(np.float32)
    b1 = inputs["b1"].astype(np.float32)
    b2 = inputs["b2"].astype(np.float32)
    b3 = inputs["b3"].astype(np.float32)

    mmnp = np.float16 if MM_DT == F16 else np.float32

    cols = np.arange(D * 23).reshape(D, 23)
    # uw/uh interleaved (k, d, q): the (cumw, cumh) pair for one dim lands in
    # adjacent f16 elements (walked as one f32); ud stays (j, d).
    wh = np.stack([cols[:, 0:8], cols[:, 8:16]], axis=-1)  # [d, k, 2]
    perm = np.concatenate([
        wh.transpose(1, 0, 2).reshape(-1),  # (k, d, q)
        cols[:, 16:23].T.reshape(-1),
    ])
    W3r = W3[:, :, perm]
    b3r = b3[:, perm]
    has_b3 = bool(np.any(b3r != 0.0))

    ctxT = np.ascontiguousarray(ctx.T.astype(mmnp))                 # [128, B]
    w1 = np.ascontiguousarray(W1.astype(mmnp))                      # [L,128,256]
    w2 = np.ascontiguousarray(
        W2.reshape(L, 2, 128, HID).astype(mmnp))                    # [L,2,128,256]
    w3 = np.ascontiguousarray(
        W3r.reshape(L, 2, 128, ODIM).astype(mmnp))                  # [L,2,128,1472]
    bias1 = np.ascontiguousarray(b1.reshape(L, 2, 128))
    bias2 = np.ascontiguousarray(b2.reshape(L, 2, 128))

    li = np.tril_indices(D, -1)
    ui = np.triu_indices(D, 1)
    at = np.zeros((L, D, D), np.float32)
    lld = 0.0
    for l in range(L):
        Lm = np.eye(D, dtype=np.float64)
        Lm[li] = inputs["lower_entries"][l].astype(np.float64)
        diag = np.log1p(np.exp(inputs["upper_diag"][l].astype(np.float64))) + 1e-3
        Um = np.zeros((D, D), np.float64)
        Um[ui] = inputs["upper_entries"][l].astype(np.float64)
        Um += np.diag(diag)
        at[l] = (Lm @ Um).T.astype(np.float32)  # cast to mm dtype below
        lld += float(np.sum(np.log(diag)))
    lub = np.ascontiguousarray(inputs["lu_bias"].astype(np.float32))

    cj = ((2.0 * BND * MIN_W) * np.arange(1, 8).astype(np.float32) - BND)
    btnp = np.float16 if BT == F16 else np.float32
    cedge = np.ascontiguousarray(
        np.broadcast_to(cj[:, None], (7, 2 * D)).astype(btnp))

    const_total = -0.5 * D * float(np.log(2.0 * np.pi)) + lld

    has_lub = bool(np.any(lub != 0.0))
    shared = dict(w1=w1, w2=w2, w3=w3, bias1=bias1, bias2=bias2,
                  at=at.astype(mmnp), cedge=cedge)
    if has_lub:
        shared["lub"] = lub
    if has_b3:
        shared["bias3"] = np.ascontiguousarray(b3r.reshape(1, ODIM))
    return x, ctxT, shared, has_b3, has_lub, const_total


_CACHE = {}


def kernel(**inputs):
    rpc = inputs["inputs"].shape[0] // N_CORES
    x, ctxT, shared, has_b3, has_lub, const_total = _host_prep(inputs)

    key = (rpc, has_b3, has_lub)
    if key not in _CACHE:
        _CACHE[key] = build_program(rpc, has_b3=has_b3, has_lub=has_lub)
    nc = _CACHE[key]

    in_maps = []
    for c in range(N_CORES):
        m = dict(shared)
        m["xin"] = np.ascontiguousarray(x[c * rpc:(c + 1) * rpc])
        m["ctxT"] = np.ascontiguousarray(ctxT[:, c * rpc:(c + 1) * rpc])
        in_maps.append(m)

    res = bass_utils.run_bass_kernel_spmd(nc, in_maps, core_ids=list(range(N_CORES)))
    out = np.concatenate([r["out"] for r in res.results])
    return (out + np.float32(const_total)).astype(np.float32)


# revision 85
# speedup vs baseline: 1.0143x; 1.0001x over previous
"""Trainium2 Bass kernel for nn_ConditionalSplineFlow (8-core data parallel).

Layout strategy:
  - MLP runs in "transposed world": activations [feature, rows] so weight
    matrices act as lhsT directly; final GEMM flips orientation using h2^T as
    lhsT, giving params [rows(part), 1472(free)] with W3 columns reordered to
    [uw(k-major,d-inner) | uh | ud(j-major)].
  - Bin space is batched per sub-chunk of RT row-tiles with uw/uh columns
    interleaved (k, d, q) so the (cumw, cumh) edge pair for one dim sits in
    adjacent f16 elements: the bin-search walk moves both via one f32-bitcast
    predicated copy (half the element count). Cumsum chain is an unrolled f16
    DVE tensor-add chain (2x perf mode), 1/S comes from ACT ln/exp
    (exp(-lnS + ln(SCL)) = SCL/S), edges are two batched f16 DVE tensor ops.
  - Spline formula is elementwise in (row, dim), chunked per sub-chunk.
  - LU layer folded to  x' = y @ (L@U)^T + b  via PE transpose + matmul.
  - Per-layer logdet of LU and the gaussian constant are folded on host.
"""
import os
import numpy as np
from contextlib import ExitStack

import concourse.bass as bass
import concourse.bacc as bacc
import concourse.tile as tile
import concourse.mybir as mybir
from concourse import bass_utils
from concourse.masks import make_identity

# Pin all activations to the one table set that covers Exp/Ln/Relu/Copy/Abs —
# the default per-function chooser ping-pongs between sets (~2.6us per swap,
# once per row-tile). Masking the other sets (order preserved, so positional
# set ids stay valid) forces a single resident table.
_PINNED_ACT_SET = "natural_log_exp_and_others"
_orig_gat = bacc.get_activation_tables


def _gat_pinned(arch):
    tabs = _orig_gat(arch)
    return {name: (fns if name == _PINNED_ACT_SET else set())
            for name, fns in tabs.items()}


bacc.get_activation_tables = _gat_pinned

F32 = mybir.dt.float32
F16 = mybir.dt.float16
U8 = mybir.dt.uint8
AF = mybir.ActivationFunctionType
OP = mybir.AluOpType

D = 64
NB = 8
L = 5
HID = 256
ODIM = 1472
BND = 5.0
MIN_W = 1e-3
MIN_D = 1e-3
SCL = 2.0 * BND * (1.0 - MIN_W * NB)      # 9.92
SPBOUND = 1.0 - MIN_D                      # softplus value at padded boundary
N_CORES = 8
BATCH = 32768

MM_DT = F16   # matmul dtype for MLP
BT = F16      # bin-space dtype (V / Gc / chain / edges)


def build_program(rpc, has_b3=False, has_lub=True, ablate=()):
    """Build the single-core program for `rpc` rows. Returns nc.

    `ablate` (dev-only; never set by kernel()): drop pipeline sections to
    isolate their HW cost. Subsets of {"exps", "binning", "formula", "lu"}.
    Later sections imply earlier ones stay: ablating "exps" also ablates
    "binning"/"formula"/"lu" (they consume its outputs)."""
    A = set(ablate)
    if "exps" in A:
        A |= {"binning"}
    if "binning" in A:
        A |= {"formula"}
    if "formula" in A:
        A |= {"lu"}
    nc = bacc.Bacc(
        "TRN2", target_bir_lowering=False, debug=False,
        enable_asserts=False, num_devices=N_CORES,
    )
    NT = rpc // 128               # row tiles
    CHW = min(rpc, int(os.environ.get("SPLINE_CHW", "1024")))  # mlp chunk rows
    NCH = rpc // CHW              # chunks
    RT_PER_CH = CHW // 128
    RT = int(os.environ.get("SPLINE_RT", "4"))   # bin sub-chunk (row tiles)
    assert RT_PER_CH % RT == 0
    NLNSCL = float(np.log(SCL))

    # ---------------- DRAM I/O ----------------
    xin_d = nc.dram_tensor("xin", [rpc, D], F32, kind="ExternalInput").ap()
    ctxT_d = nc.dram_tensor("ctxT", [128, rpc], MM_DT, kind="ExternalInput").ap()
    w1_d = nc.dram_tensor("w1", [L, 128, HID], MM_DT, kind="ExternalInput").ap()
    w2_d = nc.dram_tensor("w2", [L, 2, 128, HID], MM_DT, kind="ExternalInput").ap()
    w3_d = nc.dram_tensor("w3", [L, 2, 128, ODIM], MM_DT, kind="ExternalInput").ap()
    b1_d = nc.dram_tensor("bias1", [L, 2, 128], F32, kind="ExternalInput").ap()
    b2_d = nc.dram_tensor("bias2", [L, 2, 128], F32, kind="ExternalInput").ap()
    at_d = nc.dram_tensor("at", [L, D, D], MM_DT, kind="ExternalInput").ap()
    if has_lub:
        lub_d = nc.dram_tensor("lub", [L, D], F32, kind="ExternalInput").ap()
    ce_d = nc.dram_tensor("cedge", [7, 2 * D], BT, kind="ExternalInput").ap()
    if has_b3:
        b3_d = nc.dram_tensor("bias3", [1, ODIM], F32, kind="ExternalInput").ap()
    out_d = nc.dram_tensor("out", [rpc], F32, kind="ExternalOutput").ap()

    with tile.TileContext(nc) as tc, ExitStack() as ctx:
        # ---------------- pools ----------------
        singles = ctx.enter_context(tc.tile_pool(name="singles", bufs=1))
        wpool = ctx.enter_context(tc.tile_pool(name="wpool", bufs=2))
        hpool = ctx.enter_context(tc.tile_pool(name="hpool", bufs=2))
        binp = ctx.enter_context(tc.tile_pool(name="binp", bufs=2))
        frm = ctx.enter_context(tc.tile_pool(
            name="frm", bufs=int(os.environ.get("SPLINE_FRM_BUFS", "2"))))
        ps_par = ctx.enter_context(tc.tile_pool(
            name="ps_par", bufs=int(os.environ.get("SPLINE_PSPAR_BUFS", "1")),
            space="PSUM"))
        ps_h = ctx.enter_context(tc.tile_pool(
            name="ps_h", bufs=int(os.environ.get("SPLINE_PSH_BUFS", "1")),
            space="PSUM"))
        ps_lu = ctx.enter_context(tc.tile_pool(
            name="ps_lu", bufs=int(os.environ.get("SPLINE_PSLU_BUFS", "1")),
            space="PSUM"))

        # ---------------- resident tiles ----------------
        ident = singles.tile([128, 128], F32)
        make_identity(nc, ident)
        ctxT = singles.tile([128, rpc], MM_DT)
        for c0 in range(0, rpc, CHW):
            nc.sync.dma_start(out=ctxT[:, c0:c0 + CHW],
                              in_=ctxT_d[:, c0:c0 + CHW])
        # PE p-state warmup: ~3us of back-to-back dummy matmuls during the
        # initial DMA window so the first real GEMMs run at full clock
        NWARM = int(os.environ.get("SPLINE_WARMUP_MM", "24"))
        if NWARM:
            ps_warm = ps_h.tile([128, CHW], F32, tag="psh")
            for _ in range(NWARM):
                nc.tensor.matmul(ps_warm[:, 0:128], lhsT=ident, rhs=ident)
        # x ping-pong, resident across a layer
        xa = singles.tile([128, NT, D], F32, tag="xa")
        xb = singles.tile([128, NT, D], F32, tag="xb")
        nc.sync.dma_start(out=xa, in_=xin_d.rearrange("(t p) d -> p t d", p=128))
        # per-dim logdet accumulator [128, NT, D]
        ldacc = singles.tile([128, NT, D], F32, tag="ldacc")
        if A:
            nc.vector.memset(ldacc, 0.0)
        # bias const for invs = exp(-lnS + ln(SCL))
        lnscl_t = singles.tile([128, 1], F32)
        nc.vector.memset(lnscl_t, NLNSCL)
        # edge constants c_j, duplicated per (d, q) pair: [128, 7, 128]
        cedge = singles.tile([128, 7, 2 * D], BT)
        nc.sync.dma_start(
            out=cedge,
            in_=bass.AP(tensor=ce_d.tensor, offset=0,
                        ap=[[0, 128], [2 * D, 7], [1, 2 * D]]),
        )
        if has_b3:
            ones1 = singles.tile([1, 128], F32)
            nc.vector.memset(ones1, 1.0)
            b3sb = singles.tile([1, ODIM], F32)
            nc.sync.dma_start(out=b3sb, in_=b3_d)
        # persistent V tiles: 6 packed f16 lanes per (slot j, dim d):
        # (e_j, ch_j, e_{j+1}, ch_{j+1}, sp_j, sp_{j+1}) = 3 f32 pairs, so the
        # walk moves left+right values of all planes in ONE f32-bitcast
        # predicated copy. Slots 0..7 (bin index k). Boundary constants
        # written once.
        NVB = int(os.environ.get("SPLINE_NVB", "2"))
        V6bufs = [singles.tile([128, RT, 8, 3, 2 * D], BT, name=f"V6b{i}")
                  for i in range(NVB)]
        for i in range(NVB):
            v = V6bufs[i].rearrange("p r s t (d q) -> p r s t d q", q=2)
            nc.gpsimd.memset(v[:, :, 0, 0, :, :], -BND)
            nc.gpsimd.memset(v[:, :, 7, 1, :, :], BND)
            nc.gpsimd.memset(v[:, :, 0, 2, :, 0], SPBOUND)
            nc.gpsimd.memset(v[:, :, 7, 2, :, 1], SPBOUND)

        x_cur, x_nxt = xa, xb

        for l in range(L):
            # ---------------- layer weights ----------------
            w1t = wpool.tile([128, HID], MM_DT, tag="w1")
            w2t = wpool.tile([128, 2, HID], MM_DT, tag="w2")
            w3t = wpool.tile([128, 2, ODIM], MM_DT, tag="w3")
            b1t = wpool.tile([128, 2], F32, tag="b1")
            b2t = wpool.tile([128, 2], F32, tag="b2")
            att = wpool.tile([64, D], MM_DT, tag="at")
            if has_lub:
                lubt = wpool.tile([128, D], F32, tag="lub")
            nc.sync.dma_start(out=w1t, in_=w1_d[l])
            nc.sync.dma_start(out=w2t, in_=w2_d[l].rearrange("k p h -> p k h"))
            nc.sync.dma_start(out=w3t, in_=w3_d[l].rearrange("k p h -> p k h"))
            nc.sync.dma_start(out=b1t, in_=b1_d[l].rearrange("t p -> p t"))
            nc.sync.dma_start(out=b2t, in_=b2_d[l].rearrange("t p -> p t"))
            nc.sync.dma_start(out=att, in_=at_d[l])
            if has_lub:
                nc.sync.dma_start(
                    out=lubt,
                    in_=bass.AP(tensor=lub_d.tensor, offset=l * D,
                                ap=[[0, 128], [1, D]]),
                )

            for chi in range(NCH):
                # ---------------- MLP chunk (transposed world) ----------------
                c0 = chi * CHW
                NHALF = max(1, CHW // 512)
                h1t = hpool.tile([128, 2, CHW], MM_DT, tag="h1")
                for m in range(2):
                    ps1 = ps_h.tile([128, CHW], F32, tag="psh")
                    for hf in range(NHALF):
                        h0 = hf * 512
                        hw_ = min(512, CHW - h0)
                        nc.tensor.matmul(
                            ps1[:, h0:h0 + hw_],
                            lhsT=w1t[:, m * 128:(m + 1) * 128],
                            rhs=ctxT[:, c0 + h0:c0 + h0 + hw_])
                    nc.scalar.activation(h1t[:, m, :], ps1, AF.Relu,
                                         bias=b1t[:, m:m + 1])
                h2t = hpool.tile([128, 2, CHW], MM_DT, tag="h2")
                for m in range(2):
                    ps2 = ps_h.tile([128, CHW], F32, tag="psh")
                    for hf in range(NHALF):
                        h0 = hf * 512
                        hw_ = min(512, CHW - h0)
                        for kk in range(2):
                            nc.tensor.matmul(
                                ps2[:, h0:h0 + hw_],
                                lhsT=w2t[:, kk, m * 128:(m + 1) * 128],
                                rhs=h1t[:, kk, h0:h0 + hw_],
                                start=(kk == 0), stop=(kk == 1))
                    nc.scalar.activation(h2t[:, m, :], ps2, AF.Relu,
                                         bias=b2t[:, m:m + 1])

                if "binning" not in A:
                    xcc_nxt = frm.tile([128, RT, D], BT, tag="xcc", name="xcc")
                    nc.gpsimd.tensor_scalar(
                        xcc_nxt,
                        x_cur[:, chi * RT_PER_CH:chi * RT_PER_CH + RT, :],
                        -BND, BND, OP.max, OP.min)
                for sub in range(RT_PER_CH // RT):
                    rt0 = chi * RT_PER_CH + sub * RT
                    V6 = V6bufs[(rt0 // RT) % NVB]
                    if "exps" not in A:
                        # exp(params) tile, one ACT write per row-tile:
                        # cols 0:1024 = uw/uh [8(k), 64(d), 2(q)] (cumsummed in
                        # place below), cols 1024:1472 = ud [7, 64].
                        EP = binp.tile([128, RT, ODIM], BT, tag="EP", name="EP")
                        E = EP[:, :, 0:1024].rearrange(
                            "p r (k d q) -> p r k d q", k=NB, d=D, q=2)
                        EU = EP[:, :, 1024:1472].rearrange(
                            "p r (j d) -> p r j d", j=7, d=D)
                    if "binning" not in A:
                        Gc6 = binp.tile([128, RT, 3, 2 * D], BT, tag="Gc6",
                                        name="Gc6")
                        xcc = xcc_nxt
                        if sub + 1 < RT_PER_CH // RT:
                            # prefetch next sub-chunk's clip ahead of this
                            # sub-chunk's formula ops in Pool's in-order queue
                            xcc_nxt = frm.tile([128, RT, D], BT, tag="xcc",
                                               name="xcc")
                            nc.gpsimd.tensor_scalar(
                                xcc_nxt, x_cur[:, rt0 + RT:rt0 + 2 * RT, :],
                                -BND, BND, OP.max, OP.min)
                    if A:
                        rq = frm.tile([128, RT], F32, tag="rq", name="rq")

                    for rti in range(RT):
                        rt = rt0 + rti
                        r0 = (sub * RT + rti) * 128
                        # -------- GEMM3: params [128 rows, 1472] --------
                        psp = ps_par.tile([128, ODIM], F32, tag="pspar")
                        nslices = [(0, 512), (512, 512), (1024, 448)]
                        for (ns, nw) in nslices:
                            for kk in range(2):
                                nc.tensor.matmul(
                                    psp[:, ns:ns + nw],
                                    lhsT=h2t[:, kk, r0:r0 + 128],
                                    rhs=w3t[:, kk, ns:ns + nw],
                                    start=(kk == 0),
                                    stop=(kk == 1) if not has_b3 else False,
                                )
                            if has_b3:
                                nc.tensor.matmul(
                                    psp[:, ns:ns + nw], lhsT=ones1,
                                    rhs=b3sb[:, ns:ns + nw],
                                    start=False, stop=True)

                        if "exps" in A:
                            nc.vector.tensor_reduce(
                                rq[:, rti:rti + 1],
                                psp[:, 0:64].unsqueeze(1),
                                mybir.AxisListType.X, OP.add)
                            continue
                        # -------- exp / softplus (ACT) --------
                        nc.scalar.activation(EP[:, rti], psp, AF.Exp)
                        # softplus(ud) = Ln(exp(ud) + 1) into BOTH sp lane
                        # destinations with one ACT op: the last AP dim strides
                        # from (slot j-1, lane 1) to (slot j, lane 0), and the
                        # input reads each value twice via a stride-0 dim.
                        vb = V6[:, rti]
                        eu = EU[:, rti]
                        sp_out = bass.AP(
                            tensor=vb.tensor,
                            offset=vb.offset + 2 * 2 * D + 1,
                            ap=[list(vb.ap[0]), [3 * 2 * D, 7], [2, D],
                                [3 * 2 * D - 1, 2]])
                        sp_in = bass.AP(
                            tensor=eu.tensor, offset=eu.offset,
                            ap=[list(eu.ap[0]), [D, 7], [1, D], [0, 2]])
                        nc.scalar.activation(sp_out, sp_in, AF.Ln, bias=1.0)

                    if "binning" in A:
                        nc.vector.tensor_reduce(
                            rq, E[:, :, 0, :, 0], mybir.AxisListType.X, OP.add)
                        if l == 0:
                            nc.vector.tensor_copy(ldacc[:, rt0:rt0 + RT, 0:1],
                                                  rq.unsqueeze(2))
                        else:
                            nc.vector.tensor_add(
                                ldacc[:, rt0:rt0 + RT, 0:1], rq.unsqueeze(2),
                                ldacc[:, rt0:rt0 + RT, 0:1])
                        continue

                    # -------- cumsum chain, in place on E (f16, DVE) --------
                    Em = E.rearrange("p r k d q -> p r k (d q)")
                    for j in range(1, NB):
                        nc.vector.tensor_add(Em[:, :, j, :], Em[:, :, j - 1, :],
                                             Em[:, :, j, :])
                    # -------- invs = SCL / S via ACT ln/exp --------
                    lnS = frm.tile([128, RT, 2 * D], F32, tag="lnS", name="lnS")
                    nc.scalar.activation(lnS, Em[:, :, NB - 1, :], AF.Ln)
                    invs = frm.tile([128, RT, 2 * D], BT, tag="invs",
                                    name="invs")
                    nc.scalar.activation(invs, lnS, AF.Exp, bias=lnscl_t,
                                         scale=-1.0)
                    # -------- edges into packed V pair 0 (f16, DVE) --------
                    V6p0 = V6[:, :, 1:8, 0, :]
                    nc.vector.tensor_mul(
                        V6p0, Em[:, :, 0:7, :],
                        invs.unsqueeze(2).to_broadcast([128, RT, 7, 2 * D]))
                    nc.vector.tensor_add(
                        V6p0, V6p0,
                        cedge.unsqueeze(1).to_broadcast([128, RT, 7, 2 * D]))
                    # right-neighbor (e, ch) duplicated into pair 1 (TC 4x)
                    nc.vector.tensor_copy(V6[:, :, 0:7, 1, :], V6p0)

                    # -------- masks (f16, 2x) + single packed walk (DVE) -----
                    su = binp.tile([128, RT, 7, D], BT, tag="su", name="su")
                    nc.vector.tensor_tensor(
                        su,
                        xcc.unsqueeze(2).to_broadcast([128, RT, 7, D]),
                        V6[:, :, 1:8, 0, :].rearrange(
                            "p r s (d q) -> p r s d q", q=2)[:, :, :, :, 0],
                        OP.is_ge)
                    V632 = V6.bitcast(F32)      # [128, RT, 8, 3, 64]
                    Gc632 = Gc6.bitcast(F32)    # [128, RT, 3, 64]
                    nc.vector.tensor_copy(
                        Gc6.rearrange("p r t m -> p r (t m)"),
                        V6[:, :, 0, :, :].rearrange("p r t m -> p r (t m)"))
                    su16 = su.bitcast(mybir.dt.uint16)
                    for j in range(1, 8):
                        nc.vector.copy_predicated(
                            Gc632,
                            su16[:, :, j - 1:j, :].to_broadcast(
                                [128, RT, 3, D]),
                            V632[:, :, j, :, :])

                    if "formula" in A:
                        nc.vector.tensor_reduce(
                            rq,
                            Gc6[:, :, 0, :].rearrange(
                                "p r (d q) -> p r d q", q=2)[:, :, :, 0],
                            mybir.AxisListType.X, OP.add)
                        if l == 0:
                            nc.vector.tensor_copy(ldacc[:, rt0:rt0 + RT, 0:1],
                                                  rq.unsqueeze(2))
                        else:
                            nc.vector.tensor_add(
                                ldacc[:, rt0:rt0 + RT, 0:1], rq.unsqueeze(2),
                                ldacc[:, rt0:rt0 + RT, 0:1])
                        continue
                    # ------------- formula (chunked, mostly f16) -------------
                    FSH = [128, RT, D]
                    G0 = Gc6[:, :, 0, :].rearrange("p r (d q) -> p r d q", q=2)
                    G1 = Gc6[:, :, 1, :].rearrange("p r (d q) -> p r d q", q=2)
                    le, lch = G0[:, :, :, 0], G0[:, :, :, 1]
                    re_, rch = G1[:, :, :, 0], G1[:, :, :, 1]
                    Gdp = frm.tile([128, RT, 2 * D], BT, tag="Gdp", name="Gdp")
                    nc.vector.tensor_scalar(Gdp, Gc6[:, :, 2, :],
                                            MIN_D, None, OP.add)
                    Gdpq = Gdp.rearrange("p r (d q) -> p r d q", q=2)
                    ind, ind1 = Gdpq[:, :, :, 0], Gdpq[:, :, :, 1]

                    def ft(tag, dt_=BT):
                        return frm.tile(FSH, dt_, tag=tag, name=tag)

                    xt = x_cur[:, rt0:rt0 + RT, :]
                    # 1/in_w and 1/denom via ACT ln/exp keeps the mults f16-2x
                    in_w = ft("in_w"); nc.gpsimd.tensor_sub(in_w, re_, le)
                    lnw = ft("lnw")
                    nc.scalar.activation(lnw, in_w, AF.Ln)
                    rw = ft("rw")
                    nc.scalar.activation(rw, lnw, AF.Exp, scale=-1.0)
                    tnum = ft("tnum"); nc.gpsimd.tensor_sub(tnum, xcc, le)
                    th = ft("th"); nc.vector.tensor_mul(th, tnum, rw)
                    in_h = ft("in_h"); nc.gpsimd.tensor_sub(in_h, rch, lch)
                    idel = ft("idel"); nc.vector.tensor_mul(idel, in_h, rw)
                    # squares on ACT; (1-th)^2 fused via scale/bias
                    th2 = ft("th2"); nc.scalar.activation(th2, th, AF.Square)
                    omt2 = ft("omt2")
                    nc.scalar.activation(omt2, th, AF.Square, bias=1.0, scale=-1.0)
                    idel2 = ft("idel2")
                    nc.scalar.activation(idel2, idel, AF.Square)
                    tomt = ft("tomt"); nc.vector.tensor_sub(tomt, th, th2)
                    t1 = ft("t1"); nc.vector.tensor_mul(t1, idel, th2)
                    t2 = ft("t2"); nc.vector.tensor_mul(t2, ind, tomt)
                    nsum = t1; nc.vector.tensor_add(nsum, t1, t2)
                    numer = in_h; nc.vector.tensor_mul(numer, in_h, nsum)
                    dd = ft("dd"); nc.vector.tensor_add(dd, ind, ind1)
                    dd2 = ft("dd2")
                    nc.vector.scalar_tensor_tensor(dd2, idel, -2.0, dd,
                                                   OP.mult, OP.add)
                    dt = dd2; nc.vector.tensor_mul(dt, dd2, tomt)
                    denom = ft("denom")
                    nc.gpsimd.tensor_add(denom, idel, dt)
                    lnden = ft("lnden")
                    nc.scalar.activation(lnden, denom, AF.Ln)
                    rden = ft("rden")
                    nc.scalar.activation(rden, lnden, AF.Exp, scale=-1.0)
                    yq = rden; nc.vector.tensor_mul(yq, numer, rden)
                    y = ft("y", F32); nc.gpsimd.tensor_add(y, lch, yq)
                    u1 = ft("u1"); nc.vector.tensor_mul(u1, th2, ind1)
                    idt = ft("idt"); nc.vector.tensor_mul(idt, idel, tomt)
                    u2 = u1
                    nc.vector.scalar_tensor_tensor(u2, idt, 2.0, u1,
                                                   OP.mult, OP.add)
                    u3 = ft("u3"); nc.gpsimd.tensor_mul(u3, ind, omt2)
                    uu = u2; nc.vector.tensor_add(uu, u2, u3)
                    dnum = idel2; nc.vector.tensor_mul(dnum, uu, idel2)
                    lnd = ft("lnd"); nc.scalar.activation(lnd, dnum, AF.Ln)
                    ldt = ft("ldt")
                    nc.vector.scalar_tensor_tensor(ldt, lnden, -2.0, lnd,
                                                   OP.mult, OP.add)
                    # inside mask + select (f16 mask, bitcast for predication)
                    absx = ft("absx")
                    nc.scalar.activation(absx, xt, AF.Abs)
                    insu = frm.tile([128, RT, D], BT, tag="insu", name="insu")
                    nc.vector.tensor_scalar(insu, absx, BND, None, OP.is_le)
                    yfin = ft("yfin", F32)
                    nc.gpsimd.tensor_copy(yfin, xt)
                    nc.vector.copy_predicated(yfin, insu.bitcast(mybir.dt.uint16),
                                              y)
                    # masked per-dim logdet accumulate, chained across layers
                    ldm = ft("ldm")
                    nc.vector.tensor_mul(ldm, ldt, insu)
                    if l == 0:
                        nc.gpsimd.tensor_copy(ldacc[:, rt0:rt0 + RT, :], ldm)
                    else:
                        nc.gpsimd.tensor_add(ldacc[:, rt0:rt0 + RT, :], ldm,
                                             ldacc[:, rt0:rt0 + RT, :])

                    # -------- LU per row-tile: x' = y @ A^T + b --------
                    if "lu" in A:
                        continue
                    for rti in range(RT):
                        rt = rt0 + rti
                        pst = ps_lu.tile([64, 128], F32, tag="pst")
                        nc.tensor.transpose(pst, yfin[:, rti, :], ident)
                        yT = frm.tile([64, 128], MM_DT, tag="yT", name="yT")
                        nc.scalar.copy(yT, pst)
                        psx = ps_lu.tile([128, D], F32, tag="psx")
                        nc.tensor.matmul(psx, lhsT=yT, rhs=att)
                        if has_lub:
                            nc.vector.tensor_add(x_nxt[:, rt, :], psx, lubt)
                        else:
                            nc.scalar.copy(x_nxt[:, rt, :], psx)


            if "lu" not in A:
                x_cur, x_nxt = x_nxt, x_cur

        # ---------------- final: out = -0.5*sum(x^2) + sum_d ld + const -----
        xsq = singles.tile([128, NT, D], F32)
        nc.vector.tensor_mul(xsq, x_cur, x_cur)
        ov2 = singles.tile([128, NT, D], F32)
        nc.vector.scalar_tensor_tensor(ov2, xsq, -0.5, ldacc, OP.mult, OP.add)
        ov = singles.tile([128, NT], F32)
        nc.vector.tensor_reduce(ov, ov2, mybir.AxisListType.X, OP.add)
        # const added on host (exact); DMA out
        nc.sync.dma_start(out=out_d.rearrange("(t p) -> p t", p=128), in_=ov)

    nc.compile()
    return nc


# ------------------------- host side -------------------------

def _host_prep(inputs):
    x = np.ascontiguousarray(inputs["inputs"].astype(np.float32))
    ctx = inputs["context"].astype(np.float32)
    W1 = inputs["W1"].astype(np.float32)
    W2 = inputs["W2"].astype(np.float32)
    W3 = inputs["W3"].astype(np.float32)
    b1 = inputs["b1"].astype(np.float32)
    b2 = inputs["b2"].astype(np.float32)
    b3 = inputs["b3"].astype(np.float32)

    mmnp = np.float16 if MM_DT == F16 else np.float32

    cols = np.arange(D * 23).reshape(D, 23)
    # uw/uh interleaved (k, d, q): the (cumw, cumh) pair for one dim lands in
    # adjacent f16 elements (walked as one f32); ud stays (j, d).
    wh = np.stack([cols[:, 0:8], cols[:, 8:16]], axis=-1)  # [d, k, 2]
    perm = np.concatenate([
        wh.transpose(1, 0, 2).reshape(-1),  # (k, d, q)
        cols[:, 16:23].T.reshape(-1),
    ])
    W3r = W3[:, :, perm]
    b3r = b3[:, perm]
    has_b3 = bool(np.any(b3r != 0.0))

    ctxT = np.ascontiguousarray(ctx.T.astype(mmnp))                 # [128, B]
    w1 = np.ascontiguousarray(W1.astype(mmnp))                      # [L,128,256]
    w2 = np.ascontiguousarray(
        W2.reshape(L, 2, 128, HID).astype(mmnp))                    # [L,2,128,256]
    w3 = np.ascontiguousarray(
        W3r.reshape(L, 2, 128, ODIM).astype(mmnp))                  # [L,2,128,1472]
    bias1 = np.ascontiguousarray(b1.reshape(L, 2, 128))
    bias2 = np.ascontiguousarray(b2.reshape(L, 2, 128))

    li = np.tril_indices(D, -1)
    ui = np.triu_indices(D, 1)
    at = np.zeros((L, D, D), np.float32)
    lld = 0.0
    for l in range(L):
        Lm = np.eye(D, dtype=np.float64)
        Lm[li] = inputs["lower_entries"][l].astype(np.float64)
        diag = np.log1p(np.exp(inputs["upper_diag"][l].astype(np.float64))) + 1e-3
        Um = np.zeros((D, D), np.float64)
        Um[ui] = inputs["upper_entries"][l].astype(np.float64)
        Um += np.diag(diag)
        at[l] = (Lm @ Um).T.astype(np.float32)  # cast to mm dtype below
        lld += float(np.sum(np.log(diag)))
    lub = np.ascontiguousarray(inputs["lu_bias"].astype(np.float32))

    cj = ((2.0 * BND * MIN_W) * np.arange(1, 8).astype(np.float32) - BND)
    btnp = np.float16 if BT == F16 else np.float32
    cedge = np.ascontiguousarray(
        np.broadcast_to(cj[:, None], (7, 2 * D)).astype(btnp))

    const_total = -0.5 * D * float(np.log(2.0 * np.pi)) + lld

    has_lub = bool(np.any(lub != 0.0))
    shared = dict(w1=w1, w2=w2, w3=w3, bias1=bias1, bias2=bias2,
                  at=at.astype(mmnp), cedge=cedge)
    if has_lub:
        shared["lub"] = lub
    if has_b3:
        shared["bias3"] = np.ascontiguousarray(b3r.reshape(1, ODIM))
    return x, ctxT, shared, has_b3, has_lub, const_total


_CACHE = {}


def kernel(**inputs):
    rpc = inputs["inputs"].shape[0] // N_CORES
    x, ctxT, shared, has_b3, has_lub, const_total = _host_prep(inputs)

    key = (rpc, has_b3, has_lub)
    if key not in _CACHE:
        _CACHE[key] = build_program(rpc, has_b3=has_b3, has_lub=has_lub)
    nc = _CACHE[key]

    in_maps = []
    for c in range(N_CORES):
        m = dict(shared)
        m["xin"] = np.ascontiguousarray(x[c * rpc:(c + 1) * rpc])
        m["ctxT"] = np.ascontiguousarray(ctxT[:, c * rpc:(c + 1) * rpc])
        in_maps.append(m)

    res = bass_utils.run_bass_kernel_spmd(nc, in_maps, core_ids=list(range(N_CORES)))
    out = np.concatenate([r["out"] for r in res.results])
    return (out + np.float32(const_total)).astype(np.float32)
